# revision 30
# baseline (speedup 1.0000x reference)
"""AnisotropySuppressionLoss on 8 TRN2 NeuronCores (Bass/Tile), v2.

Per image (1024x1024 fp32):
  s1: A[m, v] = DFT_u x[u, m], v = 0..512 (real-input half spectrum),
      via f32r matmuls. Host pre-permutes image columns to [even m | odd m]
      so A lands in even/odd m' blocks.
  s2: radix-2 over m with the twiddle absorbed into the odd-m DFT matrix:
      Fe = DFT_{even m}(A), Fo' = sum_{odd m} A e^{-2pi i m w/1024};
      Z(w) = Fe + Fo', Z(w+512) = Fe - Fo' (w = 0..511). Halves matmul work
      vs a direct 1024-wide second DFT; butterflies are 4 DVE adds/block.
  power: U+/- = w_v/H^2 * |Z|^2 via ACT squares (scale) + DVE stt squares.
  fold:  G[v,w'] = U+(w') + U-(512-w') into bf16 strips; diagonal fold
      G + G^T via PE transpose accumulated in PSUM (ACT copy back).
  radial: barrel-shift shear (exact), bf16, 8 rounds/strip.
  loss_img = sum w*P^2/w - sum_b S_b^2/c_b  (+ H*W*eps^2 on host).
Data-parallel: batch 16 -> 2 images/core on 8 cores; host averages.
"""

import os
import sys

sys.path.insert(0, "/opt/trn_rl_repo")

import numpy as np

import concourse.bass as bass
import concourse.tile as tile
from concourse import bacc, mybir
from concourse.bass_utils import run_bass_kernel_spmd
from concourse.masks import make_identity

F32 = mybir.dt.float32
F32R = mybir.dt.float32r
BF16 = mybir.dt.bfloat16

H = 1024
NQ = 513          # quadrant size (|du|, |dv| in 0..512)
NB = 725          # radial bins 0..724
WB = 728          # barrel buffer width
NROUNDS = 8
N_CORES = 8
IMGS_PER_CORE = 2
WA = 0.002
EPS = 1e-12
CHUNKS = [(0, 128), (128, 256), (256, 384), (384, 512), (512, 513)]
AF = mybir.ActivationFunctionType

_CACHE = {}


# ---------------------------------------------------------------- host consts
def _gen_barrel_masks():
    """Per (chunk, round): (lo, hi, move_mask[128, hi-lo]) in quadrant coords.
    Cells (a, b>=a) carry delta = bin - b; each round moves cells with bit t
    set right by 2^t. Merges are exact (same remaining delta)."""
    rem = -np.ones((NQ, WB), dtype=np.int64)
    for a in range(NQ):
        cols = np.arange(a, NQ)
        bins = np.floor(np.sqrt(a * a + cols.astype(np.float64) ** 2)).astype(np.int64)
        rem[a, cols] = bins - cols
    table = [[] for _ in CHUNKS]
    for t in range(NROUNDS):
        bit = 1 << t
        move = (rem >= 0) & ((rem & bit) != 0)
        for ci, (c0, c1) in enumerate(CHUNKS):
            mv = move[c0:c1]
            cols_any = np.nonzero(mv.any(axis=0))[0]
            if len(cols_any) == 0:
                table[ci].append((0, 0, None))
            else:
                # widen span to even bounds: 4B-aligned bf16 slices let the
                # DVE pick its 2x packed mode
                lo, hi = int(cols_any[0]) & ~1, int(cols_any[-1]) + 1
                hi += hi & 1
                m = np.zeros((128, hi - lo), dtype=np.float32)
                m[: c1 - c0] = mv[:, lo:hi]
                table[ci].append((lo, hi, m))
        new_rem = -np.ones_like(rem)
        stay = (rem >= 0) & ~move
        new_rem[stay] = rem[stay]
        sr, sc = np.nonzero(move)
        dc = sc + bit
        landing = rem[sr, sc] - bit
        cur = new_rem[sr, dc]
        assert ((cur == -1) | (cur == landing)).all()
        new_rem[sr, dc] = landing
        rem = new_rem
    assert (rem[rem >= 0] == 0).all()
    return table


def _host_constants():
    if "consts" in _CACHE:
        return _CACHE["consts"]
    import ml_dtypes

    u = np.arange(H, dtype=np.float64)
    v = np.arange(520, dtype=np.float64)
    ang1 = 2.0 * np.pi * np.outer(u, v) / H
    Cm = np.cos(ang1).astype(np.float32)        # [1024, 520]
    Sm = np.sin(ang1).astype(np.float32)

    mp = np.arange(512, dtype=np.float64)
    w = np.arange(512, dtype=np.float64)
    ae = 2.0 * np.pi * np.outer(mp, w) / 512.0
    ao = 2.0 * np.pi * np.outer(2 * mp + 1, w) / 1024.0
    Ce = np.cos(ae).astype(np.float32)
    Se = np.sin(ae).astype(np.float32)
    Co = np.cos(ao).astype(np.float32)
    So = np.sin(ao).astype(np.float32)

    # radial bin counts exactly as reference._radial_bins (unshifted coords)
    y = np.minimum(np.arange(H), H - np.arange(H))
    yy, xx = np.meshgrid(y, y, indexing="ij")
    dist = np.sqrt((xx.astype(np.float64)) ** 2 + yy.astype(np.float64) ** 2)
    bins_full = np.clip(dist.astype(np.int32), 0, NB - 1)
    counts = np.bincount(bins_full.reshape(-1), minlength=NB).astype(np.float64)
    invc = np.zeros((1, WB), dtype=np.float32)
    invc[0, :NB] = (1.0 / counts).astype(np.float32)

    # row weights w_v for v = 0..512; sw cols: 2*mu = sqrt(w)/H (ACT square
    # scale), 2*mu+1 = 1/w (p2 accumulation), 10+mu = w/H^2 (DVE stt square)
    wv = np.full(NQ, 2.0)
    wv[0] = 1.0
    wv[512] = 1.0
    swc = np.zeros((128, 16), dtype=np.float32)
    for mu in range(5):
        c0, c1 = CHUNKS[mu]
        n = c1 - c0
        swc[:n, 2 * mu] = (np.sqrt(wv[c0:c1]) / H).astype(np.float32)
        swc[:n, 2 * mu + 1] = (1.0 / wv[c0:c1]).astype(np.float32)
        swc[:n, 10 + mu] = (wv[c0:c1] / (H * H)).astype(np.float32)

    table = _gen_barrel_masks()
    chunk_w = [max(1, sum(hi - lo for (lo, hi, m) in table[ci])) for ci in range(5)]
    maxw = max(chunk_w)
    bmask = np.zeros((640, maxw), dtype=np.float32)
    for ci in range(5):
        off = 0
        for (lo, hi, m) in table[ci]:
            if m is None:
                continue
            bmask[128 * ci : 128 * ci + 128, off : off + hi - lo] = m
            off += hi - lo

    # diagonal-block mask: 0 below diag, 0.5 on diag, 1 above (block-local)
    a = np.arange(128)
    mfd = (a[None, :] > a[:, None]).astype(np.float32)
    mfd[a, a] = 0.5

    _CACHE["consts"] = dict(
        Cm=Cm, Sm=Sm, Ce=Ce, Se=Se, Sen=(-Se), Co=Co, So=So, Son=(-So),
        invc=invc, swc=swc,
        bmask_bf16=bmask.astype(ml_dtypes.bfloat16),
        mfd_bf16=mfd.astype(ml_dtypes.bfloat16),
        table=table, maxw=maxw, counts=counts,
    )
    return _CACHE["consts"]


# ---------------------------------------------------------------- device build
def _build_nc():
    hc = _host_constants()
    table, maxw = hc["table"], hc["maxw"]

    nc = bacc.Bacc("TRN2", target_bir_lowering=False, debug=False)
    x_p = nc.declare_dram_parameter("x", [IMGS_PER_CORE, H, H], F32R, isOutput=False)
    cm_p = nc.declare_dram_parameter("cm", [H, 520], F32R, isOutput=False)
    sm_p = nc.declare_dram_parameter("sm", [H, 520], F32R, isOutput=False)
    ce_p = nc.declare_dram_parameter("ce", [512, 512], F32R, isOutput=False)
    se_p = nc.declare_dram_parameter("se", [512, 512], F32R, isOutput=False)
    sen_p = nc.declare_dram_parameter("sen", [512, 512], F32R, isOutput=False)
    co_p = nc.declare_dram_parameter("co", [512, 512], F32R, isOutput=False)
    so_p = nc.declare_dram_parameter("so", [512, 512], F32R, isOutput=False)
    son_p = nc.declare_dram_parameter("son", [512, 512], F32R, isOutput=False)
    bm_p = nc.declare_dram_parameter("bm", [640, maxw], BF16, isOutput=False)
    md_p = nc.declare_dram_parameter("md", [128, 128], BF16, isOutput=False)
    sw_p = nc.declare_dram_parameter("sw", [128, 16], F32, isOutput=False)
    ic_p = nc.declare_dram_parameter("ic", [1, WB], F32, isOutput=False)
    out_p = nc.declare_dram_parameter("out", [1, IMGS_PER_CORE], F32, isOutput=True)

    AT = mybir.AluOpType

    with tile.TileContext(nc) as tc:
        with (
            tc.tile_pool(name="const", bufs=1) as cpool,
            tc.tile_pool(name="xin", bufs=1) as xpool,
            tc.tile_pool(name="arr", bufs=1) as apool,
            tc.tile_pool(name="quad", bufs=1) as qpool,
            tc.tile_pool(name="work", bufs=2) as wpool,
            tc.tile_pool(name="ps", bufs=2, space="PSUM") as ps,
        ):
            # ---------------- constants
            Cm_t = [cpool.tile([128, 520], F32R, tag=f"cm{k}", name=f"cm{k}") for k in range(8)]
            Sm_t = [cpool.tile([128, 520], F32R, tag=f"sm{k}", name=f"sm{k}") for k in range(8)]
            CeT = [cpool.tile([128, 512], F32R, tag=f"ce{k}", name=f"ce{k}") for k in range(4)]
            SeT = [cpool.tile([128, 512], F32R, tag=f"sE{k}", name=f"sE{k}") for k in range(4)]
            SenT = [cpool.tile([128, 512], F32R, tag=f"sn{k}", name=f"sn{k}") for k in range(4)]
            CoT = [cpool.tile([128, 512], F32R, tag=f"co{k}", name=f"co{k}") for k in range(4)]
            SoT = [cpool.tile([128, 512], F32R, tag=f"sO{k}", name=f"sO{k}") for k in range(4)]
            SonT = [cpool.tile([128, 512], F32R, tag=f"sm{k}b", name=f"sm{k}b") for k in range(4)]
            bm_t = [
                cpool.tile([128, max(1, sum(hi - lo for (lo, hi, m) in table[ci]))],
                           BF16, tag=f"bm{ci}", name=f"bm{ci}")
                for ci in range(5)
            ]
            mfd_t = cpool.tile([128, 128], BF16, tag="mfd")
            sw_t = cpool.tile([128, 16], F32, tag="sw")
            ic_t = cpool.tile([1, WB], F32, tag="ic")
            ident = cpool.tile([128, 128], F32, tag="ident")
            make_identity(nc, ident[:])
            identr = cpool.tile([128, 128], F32R, tag="identr")
            nc.vector.tensor_copy(ident[:], ident[:])
            ones32 = cpool.tile([128, 1], F32, tag="ones32")
            nc.gpsimd.memset(ones32[:], 1.0)
            ones = cpool.tile([128, 1], F32R, tag="ones")
            nc.vector.tensor_copy(ones[:], ones32[:])
            onesb = cpool.tile([128, 1], BF16, tag="onesb")
            nc.vector.tensor_copy(onesb[:], ones32[:])
            zt = cpool.tile([128, 8], F32, tag="zt")
            nc.gpsimd.memset(zt[:], 0.0)
            lossv = cpool.tile([1, IMGS_PER_CORE], F32, tag="lossv")

            # ---------------- per-image persistent arrays
            Xt = [xpool.tile([128, H], F32R, tag=f"x{k}", name=f"x{k}") for k in range(8)]
            # A blocks: j=0..3 even m' chunks, j=4..7 odd m' chunks
            Ar = [apool.tile([128, NQ], F32R, tag=f"ar{j}", name=f"ar{j}") for j in range(8)]
            Ai = [apool.tile([128, NQ], F32R, tag=f"ai{j}", name=f"ai{j}") for j in range(8)]
            # f32r folded-G strips (shared across images) + bf16 barrel strips
            # double-buffered across images (for the pipelined red)
            Gq = [qpool.tile([128, NQ], F32, tag=f"gq{ci}", name=f"gq{ci}")
                  for ci in range(5)]
            Xb = [[qpool.tile([128, WB], BF16, tag=f"xb{p}_{ci}", name=f"xb{p}_{ci}")
                   for ci in range(5)] for p in range(IMGS_PER_CORE)]
            P2 = [qpool.tile([128, 8], F32R, tag=f"p2acc{p}", name=f"p2acc{p}")
                  for p in range(IMGS_PER_CORE)]

            def s1(img):
                """first DFT: fills Ar/Ai blocks; Ai = -Im(A)"""
                for m in range(8):
                    pr_lo = ps.tile([128, 512], F32, tag="pa")
                    pr_hi = ps.tile([128, 8], F32, tag="pd")
                    pt_lo = ps.tile([128, 512], F32, tag="pb")
                    for k in range(8):
                        lhs = Xt[k][:, 128 * m : 128 * m + 128]
                        st, sp = (k == 0), (k == 7)
                        nc.tensor.matmul(pr_lo[:], lhs, Cm_t[k][:, 0:512], start=st, stop=sp)
                        nc.tensor.matmul(pr_hi[:], lhs, Cm_t[k][:, 512:520], start=st, stop=sp)
                        nc.tensor.matmul(pt_lo[:], lhs, Sm_t[k][:, 0:512], start=st, stop=sp)
                    nc.scalar.activation(Ar[m][:, 0:512], pr_lo[:], AF.Copy)
                    nc.scalar.activation(Ar[m][:, 512:513], pr_hi[:, 0:1], AF.Copy)
                    nc.scalar.activation(Ai[m][:, 0:512], pt_lo[:], AF.Copy)
                    nc.vector.tensor_copy(Ai[m][:, 512:513], zt[:, 0:1])

            def s2pre(img):
                """zero barrel strips, the strip-4 G row, and p2acc"""
                xb = Xb[img]
                for ci in range(5):
                    nc.gpsimd.memset(xb[ci][:], 0.0)
                nc.gpsimd.memset(Gq[4][:], 0.0)
                nc.vector.tensor_copy(P2[img][:], zt[:])

            def s2row(img, mu):
                """second DFT (even/odd split) + power + fold, one v-block"""
                p2acc = P2[img]
                if True:
                    M = 128 if mu < 4 else 1
                    u0 = 128 * mu
                    pfer = ps.tile([128, 512], F32, tag="pa")
                    pfor = ps.tile([128, 512], F32, tag="pb")
                    for k in range(4):
                        st, sp = (k == 0), (k == 3)
                        er = Ar[k][:, u0 : u0 + M]
                        ei = Ai[k][:, u0 : u0 + M]
                        orr = Ar[4 + k][:, u0 : u0 + M]
                        oi = Ai[4 + k][:, u0 : u0 + M]
                        if mu < 4:
                            nc.tensor.matmul(pfer[0:M], er, CeT[k][:], start=st, stop=False)
                            nc.tensor.matmul(pfer[0:M], ei, SenT[k][:], start=False, stop=sp,
                                             skip_group_check=True)
                            nc.tensor.matmul(pfor[0:M], orr, CoT[k][:], start=st, stop=False)
                            nc.tensor.matmul(pfor[0:M], oi, SonT[k][:], start=False, stop=sp,
                                             skip_group_check=True)
                        else:
                            nc.tensor.matmul(pfer[0:M], er, CeT[k][:], start=st, stop=sp)
                            nc.tensor.matmul(pfor[0:M], orr, CoT[k][:], start=st, stop=sp)
                    sc_ap = sw_t[0:M, 2 * mu : 2 * mu + 1]
                    feR = wpool.tile([128, 512], F32, tag="feR", bufs=1)
                    zrp = wpool.tile([128, 512], F32, tag="zrp", bufs=1)
                    zrm = wpool.tile([128, 512], F32, tag="zrm", bufs=1)
                    nc.scalar.activation(feR[0:M], pfer[0:M], AF.Copy, scale=sc_ap)
                    nc.vector.scalar_tensor_tensor(
                        zrp[0:M], pfor[0:M], sc_ap, feR[0:M], op0=AT.mult, op1=AT.add)
                    nc.vector.scalar_tensor_tensor(
                        zrm[0:M], pfor[0:M], sc_ap, feR[0:M], op0=AT.mult, op1=AT.subtract)
                    pnei = ps.tile([128, 512], F32, tag="pa")
                    pnoi = ps.tile([128, 512], F32, tag="pb")
                    for k in range(4):
                        st, sp = (k == 0), (k == 3)
                        er = Ar[k][:, u0 : u0 + M]
                        ei = Ai[k][:, u0 : u0 + M]
                        orr = Ar[4 + k][:, u0 : u0 + M]
                        oi = Ai[4 + k][:, u0 : u0 + M]
                        if mu < 4:
                            nc.tensor.matmul(pnei[0:M], ei, CeT[k][:], start=st, stop=False)
                            nc.tensor.matmul(pnei[0:M], er, SeT[k][:], start=False, stop=sp,
                                             skip_group_check=True)
                            nc.tensor.matmul(pnoi[0:M], oi, CoT[k][:], start=st, stop=False)
                            nc.tensor.matmul(pnoi[0:M], orr, SoT[k][:], start=False, stop=sp,
                                             skip_group_check=True)
                        else:
                            nc.tensor.matmul(pnei[0:M], er, SeT[k][:], start=st, stop=sp)
                            nc.tensor.matmul(pnoi[0:M], orr, SoT[k][:], start=st, stop=sp)
                    feI = wpool.tile([128, 512], F32, tag="feI", bufs=1)
                    zip_ = wpool.tile([128, 512], F32, tag="zip", bufs=1)
                    zim = wpool.tile([128, 512], F32, tag="zim", bufs=1)
                    nc.scalar.activation(feI[0:M], pnei[0:M], AF.Copy, scale=sc_ap)
                    nc.vector.scalar_tensor_tensor(
                        zip_[0:M], pnoi[0:M], sc_ap, feI[0:M], op0=AT.mult, op1=AT.add)
                    nc.vector.scalar_tensor_tensor(
                        zim[0:M], pnoi[0:M], sc_ap, feI[0:M], op0=AT.mult, op1=AT.subtract)

                    # U+/- = |sc*Z|^2 (scale already folded into Z);
                    # computed in place: up aliases zrp, um aliases zrm
                    up, um = zrp, zrm
                    nc.scalar.activation(up[0:M], zrp[0:M], AF.Square)
                    nc.scalar.activation(zip_[0:M], zip_[0:M], AF.Square)
                    nc.vector.tensor_tensor(out=up[0:M], in0=up[0:M], in1=zip_[0:M], op=AT.add)
                    nc.scalar.activation(zrm[0:M], zrm[0:M], AF.Square)
                    nc.scalar.activation(zim[0:M], zim[0:M], AF.Square)
                    nc.vector.tensor_tensor(out=um[0:M], in0=um[0:M], in1=zim[0:M], op=AT.add)
                    if mu == 0:
                        # zero DC: kills the catastrophic p2/q2 cancellation
                        nc.vector.tensor_copy(up[0:1, 0:1], zt[0:1, 0:1])
                    # fold to G strip: G[w']=U+(w')+U-(512-w')
                    nc.vector.tensor_tensor(
                        out=Gq[mu][0:M, 1:512], in0=up[0:M, 1:512],
                        in1=um[0:M, 511:0:-1], op=AT.add)
                    nc.vector.tensor_copy(Gq[mu][0:M, 0:1], up[0:M, 0:1])
                    nc.vector.tensor_copy(Gq[mu][0:M, 512:513], um[0:M, 0:1])
                    # p2 accumulation
                    rsp = wpool.tile([128, 1], F32, tag="rsp")
                    rsm = wpool.tile([128, 1], F32, tag="rsm")
                    nc.scalar.activation(up[0:M], up[0:M], AF.Square, accum_out=rsp[0:M])
                    nc.scalar.activation(um[0:M], um[0:M], AF.Square, accum_out=rsm[0:M])
                    nc.vector.scalar_tensor_tensor(
                        p2acc[0:M, 0:1], rsp[0:M], sw_t[0:M, 2 * mu + 1 : 2 * mu + 2],
                        p2acc[0:M, 0:1], op0=AT.mult, op1=AT.add)
                    nc.vector.scalar_tensor_tensor(
                        p2acc[0:M, 0:1], rsm[0:M], sw_t[0:M, 2 * mu + 1 : 2 * mu + 2],
                        p2acc[0:M, 0:1], op0=AT.mult, op1=AT.add)

            def dfrow(img, ci):
                """diagonal fold G + G^T (upper triangle) via PSUM accumulate,
                writing the bf16 barrel strips directly; one destination strip"""
                xb = Xb[img]
                if True:
                    for cj in range(ci, 4):
                        tp = ps.tile([128, 128], F32, tag="pd")
                        nc.tensor.matmul(tp[:], ident[:],
                                         Gq[ci][:, 128 * cj : 128 * cj + 128],
                                         start=True, stop=False)
                        nc.tensor.matmul(tp[:], Gq[cj][:, 128 * ci : 128 * ci + 128],
                                         ident[:], is_transpose=True,
                                         start=False, stop=True,
                                         skip_group_check=True)
                        if ci == cj:
                            nc.vector.tensor_tensor(
                                out=xb[ci][:, 128 * cj : 128 * cj + 128],
                                in0=tp[:], in1=mfd_t[:], op=AT.mult)
                        else:
                            nc.scalar.activation(
                                xb[ci][:, 128 * cj : 128 * cj + 128], tp[:], AF.Copy)
                    # column 512 += transpose of strip-4 block
                    tp4 = ps.tile([128, 128], F32, tag="pd")
                    nc.tensor.matmul(tp4[:], Gq[4][:, 128 * ci : 128 * ci + 128],
                                     ident[:], is_transpose=True,
                                     start=True, stop=False)
                    nc.tensor.matmul(tp4[:, 0:1], ident[:], Gq[ci][:, 512:513],
                                     start=False, stop=True, skip_group_check=True)
                    nc.scalar.activation(xb[ci][:, 512:513], tp4[:, 0:1], AF.Copy)

            def brlstrip(img, ci):
                """barrel shear: align each row's columns to radial bins"""
                xb = Xb[img]
                if True:
                    off = 0
                    for t in range(NROUNDS):
                        lo, hi, m = table[ci][t]
                        wdt = hi - lo
                        if wdt <= 0:
                            continue
                        bit = 1 << t
                        eng = nc.gpsimd if ci >= 2 else nc.vector
                        tmp = wpool.tile([128, 640 if ci < 2 else 288], BF16,
                                         tag=("btmpg" if ci >= 2 else "btmp"), bufs=1)
                        eng.tensor_tensor(
                            out=tmp[:, 0:wdt], in0=xb[ci][:, lo:hi],
                            in1=bm_t[ci][:, off : off + wdt], op=AT.mult)
                        eng.tensor_tensor(
                            out=xb[ci][:, lo:hi], in0=xb[ci][:, lo:hi],
                            in1=tmp[:, 0:wdt], op=AT.subtract)
                        eng.tensor_tensor(
                            out=xb[ci][:, lo + bit : hi + bit],
                            in0=xb[ci][:, lo + bit : hi + bit],
                            in1=tmp[:, 0:wdt], op=AT.add)
                        off += wdt

            def red(img):
                """per-bin sums -> q2; loss = p2 - q2"""
                xb = Xb[img]
                ps_lo = ps.tile([1, 512], F32, tag="pa")
                ps_hi = ps.tile([1, 216], F32, tag="pb")
                for ci in range(5):
                    st, sp = (ci == 0), (ci == 4)
                    nc.tensor.matmul(ps_lo[:], onesb[:], xb[ci][:, 0:512], start=st, stop=sp)
                    nc.tensor.matmul(ps_hi[:], onesb[:], xb[ci][:, 512:WB], start=st, stop=sp)
                ssq = wpool.tile([1, WB], F32, tag="ssq", bufs=1)
                nc.scalar.activation(ssq[0:1, 0:512], ps_lo[:], AF.Square)
                nc.scalar.activation(ssq[0:1, 512:WB], ps_hi[:], AF.Square)
                nc.vector.tensor_tensor(out=ssq[:], in0=ssq[:], in1=ic_t[:], op=AT.mult)
                q2 = wpool.tile([1, 1], F32, tag="q2")
                nc.vector.tensor_reduce(q2[:], ssq[:], axis=mybir.AxisListType.X, op=AT.add)
                psp = ps.tile([1, 8], F32, tag="pd")
                nc.tensor.matmul(psp[:], ones[:], P2[img][:], start=True, stop=True)
                nc.vector.tensor_tensor(
                    out=lossv[0:1, img : img + 1], in0=psp[0:1, 0:1], in1=q2[:],
                    op=AT.subtract)

            def s2df(img):
                """s2 + fold + barrel, v-blocks descending so each strip's
                barrel overlaps the remaining blocks' matmuls"""
                s2pre(img)
                s2row(img, 4)
                # strip 4: the lone (512,512) diagonal cell keeps weight 1
                nc.scalar.activation(Xb[img][4][0:1, 512:513],
                                     Gq[4][0:1, 512:513], AF.Copy)
                brlstrip(img, 4)
                for mu in (3, 2, 1, 0):
                    s2row(img, mu)
                    dfrow(img, mu)
                    brlstrip(img, mu)

            # ---------------- program (software-pipelined over 2 images)
            # PE warmup: keep HAM un-throttled while the input/const DMAs
            # stream in (PE would otherwise idle ~20us and start cold)
            for i in range(96):
                pw = ps.tile([128, 128], F32, tag="pd")
                nc.tensor.matmul(pw[:], ident[:], ident[:], start=True, stop=True)

            for k in range(8):
                nc.sync.dma_start(Xt[k][:], x_p[0, 128 * k : 128 * k + 128, :])
            for k in range(8):
                nc.sync.dma_start(Cm_t[k][:], cm_p[128 * k : 128 * k + 128, :])
                nc.sync.dma_start(Sm_t[k][:], sm_p[128 * k : 128 * k + 128, :])
            nc.sync.dma_start(sw_t[:], sw_p[:])
            nc.sync.dma_start(ic_t[:], ic_p[:])
            for k in range(4):
                sl = slice(128 * k, 128 * k + 128)
                nc.sync.dma_start(CeT[k][:], ce_p[sl, :])
                nc.sync.dma_start(SeT[k][:], se_p[sl, :])
                nc.sync.dma_start(SenT[k][:], sen_p[sl, :])
                nc.sync.dma_start(CoT[k][:], co_p[sl, :])
                nc.sync.dma_start(SoT[k][:], so_p[sl, :])
                nc.sync.dma_start(SonT[k][:], son_p[sl, :])

            sc = nc.named_scope("s1_0"); sc.__enter__()
            s1(0)
            sc.__exit__(None, None, None)
            # prefetch image 1 and the barrel masks behind the s1 constants
            for k in range(8):
                nc.sync.dma_start(Xt[k][:], x_p[1, 128 * k : 128 * k + 128, :])
            for ci in range(5):
                wci = sum(hi - lo for (lo, hi, m) in table[ci])
                if wci > 0:
                    nc.sync.dma_start(bm_t[ci][:, 0:wci],
                                      bm_p[128 * ci : 128 * ci + 128, 0:wci])
            nc.sync.dma_start(mfd_t[:], md_p[:])

            sc = nc.named_scope("s2_0"); sc.__enter__()
            s2df(0)
            sc.__exit__(None, None, None)
            sc = nc.named_scope("s1_1"); sc.__enter__()
            s1(1)
            sc.__exit__(None, None, None)
            sc = nc.named_scope("red_0"); sc.__enter__()
            red(0)
            sc.__exit__(None, None, None)
            sc = nc.named_scope("s2_1"); sc.__enter__()
            s2df(1)
            sc.__exit__(None, None, None)
            sc = nc.named_scope("red_1"); sc.__enter__()
            red(1)
            sc.__exit__(None, None, None)

            nc.sync.dma_start(out_p[:], lossv[:])

    nc.compile()
    return nc


def _get_nc():
    if "nc" not in _CACHE:
        _CACHE["nc"] = _build_nc()
    return _CACHE["nc"]


# ---------------------------------------------------------------- entry point
def kernel(prob_cg: np.ndarray) -> np.ndarray:
    hc = _host_constants()
    nc = _get_nc()
    x = prob_cg[:, 0, :, :].astype(np.float32)
    # pre-permute columns to [even m | odd m] so s1 writes even/odd A blocks
    xp = np.ascontiguousarray(
        np.concatenate([x[:, :, 0::2], x[:, :, 1::2]], axis=2))
    in_maps = []
    for i in range(N_CORES):
        in_maps.append(
            dict(
                x=xp[2 * i : 2 * i + 2],
                cm=hc["Cm"], sm=hc["Sm"],
                ce=hc["Ce"], se=hc["Se"], sen=hc["Sen"],
                co=hc["Co"], so=hc["So"], son=hc["Son"],
                bm=hc["bmask_bf16"], md=hc["mfd_bf16"],
                sw=hc["swc"], ic=hc["invc"],
            )
        )
    trace = os.environ.get("AT_TRACE", "0") == "1"
    kw = {}
    if trace and os.environ.get("AT_TMPDIR"):
        kw["tmpdir"] = os.environ["AT_TMPDIR"]
    res = run_bass_kernel_spmd(nc, in_maps, core_ids=list(range(N_CORES)), trace=trace, **kw)
    if trace and res.exec_time_ns is not None:
        print(f"HW exec time: {res.exec_time_ns} ns")
        if res.per_core_scope_times:
            for kname, v in sorted(res.per_core_scope_times.items()):
                print(f"  scope {kname}: {v}")
    losses = np.concatenate([r["out"].reshape(-1) for r in res.results])
    loss = losses.mean() + (H * H) * (EPS * EPS)
    return np.float32(WA * loss)


# revision 32
# speedup vs baseline: 1.0134x; 1.0134x over previous
"""AnisotropySuppressionLoss on 8 TRN2 NeuronCores (Bass/Tile), v2.

Per image (1024x1024 fp32):
  s1: A[m, v] = DFT_u x[u, m], v = 0..512 (real-input half spectrum),
      via f32r matmuls. Host pre-permutes image columns to [even m | odd m]
      so A lands in even/odd m' blocks.
  s2: radix-2 over m with the twiddle absorbed into the odd-m DFT matrix:
      Fe = DFT_{even m}(A), Fo' = sum_{odd m} A e^{-2pi i m w/1024};
      Z(w) = Fe + Fo', Z(w+512) = Fe - Fo' (w = 0..511). Halves matmul work
      vs a direct 1024-wide second DFT; butterflies are 4 DVE adds/block.
  power: U+/- = w_v/H^2 * |Z|^2 via ACT squares (scale) + DVE stt squares.
  fold:  G[v,w'] = U+(w') + U-(512-w') into bf16 strips; diagonal fold
      G + G^T via PE transpose accumulated in PSUM (ACT copy back).
  radial: barrel-shift shear (exact), bf16, 8 rounds/strip.
  loss_img = sum w*P^2/w - sum_b S_b^2/c_b  (+ H*W*eps^2 on host).
Data-parallel: batch 16 -> 2 images/core on 8 cores; host averages.
"""

import os
import sys

sys.path.insert(0, "/opt/trn_rl_repo")

import numpy as np

import concourse.bass as bass
import concourse.tile as tile
from concourse import bacc, mybir
from concourse.bass_utils import run_bass_kernel_spmd
from concourse.masks import make_identity

F32 = mybir.dt.float32
F32R = mybir.dt.float32r
BF16 = mybir.dt.bfloat16

H = 1024
NQ = 513          # quadrant size (|du|, |dv| in 0..512)
NB = 725          # radial bins 0..724
WB = 728          # barrel buffer width
NROUNDS = 8
N_CORES = 8
IMGS_PER_CORE = 2
WA = 0.002
EPS = 1e-12
CHUNKS = [(0, 128), (128, 256), (256, 384), (384, 512), (512, 513)]
AF = mybir.ActivationFunctionType

_CACHE = {}


# ---------------------------------------------------------------- host consts
def _gen_barrel_masks():
    """Per (chunk, round): (lo, hi, move_mask[128, hi-lo]) in quadrant coords.
    Cells (a, b>=a) carry delta = bin - b; each round moves cells with bit t
    set right by 2^t. Merges are exact (same remaining delta)."""
    rem = -np.ones((NQ, WB), dtype=np.int64)
    for a in range(NQ):
        cols = np.arange(a, NQ)
        bins = np.floor(np.sqrt(a * a + cols.astype(np.float64) ** 2)).astype(np.int64)
        rem[a, cols] = bins - cols
    table = [[] for _ in CHUNKS]
    for t in range(NROUNDS):
        bit = 1 << t
        move = (rem >= 0) & ((rem & bit) != 0)
        for ci, (c0, c1) in enumerate(CHUNKS):
            mv = move[c0:c1]
            cols_any = np.nonzero(mv.any(axis=0))[0]
            if len(cols_any) == 0:
                table[ci].append((0, 0, None))
            else:
                # widen span to even bounds: 4B-aligned bf16 slices let the
                # DVE pick its 2x packed mode
                lo, hi = int(cols_any[0]) & ~1, int(cols_any[-1]) + 1
                hi += hi & 1
                m = np.zeros((128, hi - lo), dtype=np.float32)
                m[: c1 - c0] = mv[:, lo:hi]
                table[ci].append((lo, hi, m))
        new_rem = -np.ones_like(rem)
        stay = (rem >= 0) & ~move
        new_rem[stay] = rem[stay]
        sr, sc = np.nonzero(move)
        dc = sc + bit
        landing = rem[sr, sc] - bit
        cur = new_rem[sr, dc]
        assert ((cur == -1) | (cur == landing)).all()
        new_rem[sr, dc] = landing
        rem = new_rem
    assert (rem[rem >= 0] == 0).all()
    return table


def _host_constants():
    if "consts" in _CACHE:
        return _CACHE["consts"]
    import ml_dtypes

    u = np.arange(H, dtype=np.float64)
    v = np.arange(520, dtype=np.float64)
    ang1 = 2.0 * np.pi * np.outer(u, v) / H
    Cm = np.cos(ang1).astype(np.float32)        # [1024, 520]
    Sm = np.sin(ang1).astype(np.float32)

    mp = np.arange(512, dtype=np.float64)
    w = np.arange(512, dtype=np.float64)
    ae = 2.0 * np.pi * np.outer(mp, w) / 512.0
    ao = 2.0 * np.pi * np.outer(2 * mp + 1, w) / 1024.0
    Ce = np.cos(ae).astype(np.float32)
    Se = np.sin(ae).astype(np.float32)
    Co = np.cos(ao).astype(np.float32)
    So = np.sin(ao).astype(np.float32)

    # radial bin counts exactly as reference._radial_bins (unshifted coords)
    y = np.minimum(np.arange(H), H - np.arange(H))
    yy, xx = np.meshgrid(y, y, indexing="ij")
    dist = np.sqrt((xx.astype(np.float64)) ** 2 + yy.astype(np.float64) ** 2)
    bins_full = np.clip(dist.astype(np.int32), 0, NB - 1)
    counts = np.bincount(bins_full.reshape(-1), minlength=NB).astype(np.float64)
    invc = np.zeros((1, WB), dtype=np.float32)
    invc[0, :NB] = (1.0 / counts).astype(np.float32)

    # row weights w_v for v = 0..512; sw cols: 2*mu = sqrt(w)/H (ACT square
    # scale), 2*mu+1 = 1/w (p2 accumulation), 10+mu = w/H^2 (DVE stt square)
    wv = np.full(NQ, 2.0)
    wv[0] = 1.0
    wv[512] = 1.0
    swc = np.zeros((128, 16), dtype=np.float32)
    for mu in range(5):
        c0, c1 = CHUNKS[mu]
        n = c1 - c0
        swc[:n, 2 * mu] = (np.sqrt(wv[c0:c1]) / H).astype(np.float32)
        swc[:n, 2 * mu + 1] = (1.0 / wv[c0:c1]).astype(np.float32)
        swc[:n, 10 + mu] = (wv[c0:c1] / (H * H)).astype(np.float32)

    table = _gen_barrel_masks()
    chunk_w = [max(1, sum(hi - lo for (lo, hi, m) in table[ci])) for ci in range(5)]
    maxw = max(chunk_w)
    bmask = np.zeros((640, maxw), dtype=np.float32)
    for ci in range(5):
        off = 0
        for (lo, hi, m) in table[ci]:
            if m is None:
                continue
            bmask[128 * ci : 128 * ci + 128, off : off + hi - lo] = m
            off += hi - lo

    # diagonal-block mask: 0 below diag, 0.5 on diag, 1 above (block-local)
    a = np.arange(128)
    mfd = (a[None, :] > a[:, None]).astype(np.float32)
    mfd[a, a] = 0.5

    _CACHE["consts"] = dict(
        Cm=Cm, Sm=Sm, Ce=Ce, Se=Se, Sen=(-Se), Co=Co, So=So, Son=(-So),
        invc=invc, swc=swc,
        bmask_bf16=bmask.astype(ml_dtypes.bfloat16),
        mfd_bf16=mfd.astype(ml_dtypes.bfloat16),
        table=table, maxw=maxw, counts=counts,
    )
    return _CACHE["consts"]


# ---------------------------------------------------------------- device build
def _build_nc():
    hc = _host_constants()
    table, maxw = hc["table"], hc["maxw"]

    nc = bacc.Bacc("TRN2", target_bir_lowering=False, debug=False)
    x_p = nc.declare_dram_parameter("x", [IMGS_PER_CORE, H, H], F32R, isOutput=False)
    cm_p = nc.declare_dram_parameter("cm", [H, 520], F32R, isOutput=False)
    sm_p = nc.declare_dram_parameter("sm", [H, 520], F32R, isOutput=False)
    ce_p = nc.declare_dram_parameter("ce", [512, 512], F32R, isOutput=False)
    se_p = nc.declare_dram_parameter("se", [512, 512], F32R, isOutput=False)
    sen_p = nc.declare_dram_parameter("sen", [512, 512], F32R, isOutput=False)
    co_p = nc.declare_dram_parameter("co", [512, 512], F32R, isOutput=False)
    so_p = nc.declare_dram_parameter("so", [512, 512], F32R, isOutput=False)
    son_p = nc.declare_dram_parameter("son", [512, 512], F32R, isOutput=False)
    bm_p = nc.declare_dram_parameter("bm", [640, maxw], BF16, isOutput=False)
    md_p = nc.declare_dram_parameter("md", [128, 128], BF16, isOutput=False)
    sw_p = nc.declare_dram_parameter("sw", [128, 16], F32, isOutput=False)
    ic_p = nc.declare_dram_parameter("ic", [1, WB], F32, isOutput=False)
    out_p = nc.declare_dram_parameter("out", [1, IMGS_PER_CORE], F32, isOutput=True)

    AT = mybir.AluOpType

    with tile.TileContext(nc) as tc:
        with (
            tc.tile_pool(name="const", bufs=1) as cpool,
            tc.tile_pool(name="xin", bufs=1) as xpool,
            tc.tile_pool(name="arr", bufs=1) as apool,
            tc.tile_pool(name="quad", bufs=1) as qpool,
            tc.tile_pool(name="work", bufs=2) as wpool,
            tc.tile_pool(name="ps", bufs=2, space="PSUM") as ps,
        ):
            # ---------------- constants
            Cm_t = [cpool.tile([128, 520], F32R, tag=f"cm{k}", name=f"cm{k}") for k in range(8)]
            Sm_t = [cpool.tile([128, 520], F32R, tag=f"sm{k}", name=f"sm{k}") for k in range(8)]
            CeT = [cpool.tile([128, 512], F32R, tag=f"ce{k}", name=f"ce{k}") for k in range(4)]
            SeT = [cpool.tile([128, 512], F32R, tag=f"sE{k}", name=f"sE{k}") for k in range(4)]
            SenT = [cpool.tile([128, 512], F32R, tag=f"sn{k}", name=f"sn{k}") for k in range(4)]
            CoT = [cpool.tile([128, 512], F32R, tag=f"co{k}", name=f"co{k}") for k in range(4)]
            SoT = [cpool.tile([128, 512], F32R, tag=f"sO{k}", name=f"sO{k}") for k in range(4)]
            SonT = [cpool.tile([128, 512], F32R, tag=f"sm{k}b", name=f"sm{k}b") for k in range(4)]
            bm_t = [
                cpool.tile([128, max(1, sum(hi - lo for (lo, hi, m) in table[ci]))],
                           BF16, tag=f"bm{ci}", name=f"bm{ci}")
                for ci in range(5)
            ]
            mfd_t = cpool.tile([128, 128], BF16, tag="mfd")
            sw_t = cpool.tile([128, 16], F32, tag="sw")
            ic_t = cpool.tile([1, WB], F32, tag="ic")
            ident = cpool.tile([128, 128], F32, tag="ident")
            make_identity(nc, ident[:])
            identr = cpool.tile([128, 128], F32R, tag="identr")
            nc.vector.tensor_copy(ident[:], ident[:])
            ones32 = cpool.tile([128, 1], F32, tag="ones32")
            nc.gpsimd.memset(ones32[:], 1.0)
            ones = cpool.tile([128, 1], F32R, tag="ones")
            nc.vector.tensor_copy(ones[:], ones32[:])
            onesb = cpool.tile([128, 1], BF16, tag="onesb")
            nc.vector.tensor_copy(onesb[:], ones32[:])
            zt = cpool.tile([128, 8], F32, tag="zt")
            nc.gpsimd.memset(zt[:], 0.0)
            lossv = cpool.tile([1, IMGS_PER_CORE], F32, tag="lossv")

            # ---------------- per-image persistent arrays
            Xt = [xpool.tile([128, H], F32R, tag=f"x{k}", name=f"x{k}") for k in range(8)]
            # A blocks: j=0..3 even m' chunks, j=4..7 odd m' chunks
            Ar = [apool.tile([128, NQ], F32R, tag=f"ar{j}", name=f"ar{j}") for j in range(8)]
            Ai = [apool.tile([128, NQ], F32R, tag=f"ai{j}", name=f"ai{j}") for j in range(8)]
            # f32r folded-G strips (shared across images) + bf16 barrel strips
            # double-buffered across images (for the pipelined red)
            Gq = [qpool.tile([128, NQ], F32, tag=f"gq{ci}", name=f"gq{ci}")
                  for ci in range(5)]
            Xb = [[qpool.tile([128, WB], BF16, tag=f"xb{p}_{ci}", name=f"xb{p}_{ci}")
                   for ci in range(5)] for p in range(IMGS_PER_CORE)]
            P2 = [qpool.tile([128, 8], F32R, tag=f"p2acc{p}", name=f"p2acc{p}")
                  for p in range(IMGS_PER_CORE)]

            def s1(img):
                """first DFT: fills Ar/Ai blocks; Ai = -Im(A)"""
                for m in range(8):
                    pr_lo = ps.tile([128, 512], F32, tag="pa")
                    pr_hi = ps.tile([128, 8], F32, tag="pd")
                    pt_lo = ps.tile([128, 512], F32, tag="pb")
                    for k in range(8):
                        lhs = Xt[k][:, 128 * m : 128 * m + 128]
                        st, sp = (k == 0), (k == 7)
                        nc.tensor.matmul(pr_lo[:], lhs, Cm_t[k][:, 0:512], start=st, stop=sp)
                        nc.tensor.matmul(pr_hi[:], lhs, Cm_t[k][:, 512:520], start=st, stop=sp)
                        nc.tensor.matmul(pt_lo[:], lhs, Sm_t[k][:, 0:512], start=st, stop=sp)
                    nc.scalar.activation(Ar[m][:, 0:512], pr_lo[:], AF.Copy)
                    nc.scalar.activation(Ar[m][:, 512:513], pr_hi[:, 0:1], AF.Copy)
                    nc.scalar.activation(Ai[m][:, 0:512], pt_lo[:], AF.Copy)
                    nc.vector.tensor_copy(Ai[m][:, 512:513], zt[:, 0:1])

            def s2pre(img):
                """zero barrel strips, the strip-4 G row, and p2acc"""
                xb = Xb[img]
                for ci in range(5):
                    nc.gpsimd.memset(xb[ci][:], 0.0)
                nc.gpsimd.memset(Gq[4][:], 0.0)
                nc.vector.tensor_copy(P2[img][:], zt[:])

            def s2row(img, mu):
                """second DFT (even/odd split) + power + fold, one v-block"""
                p2acc = P2[img]
                if True:
                    M = 128 if mu < 4 else 1
                    u0 = 128 * mu
                    pfer = ps.tile([128, 512], F32, tag="pa")
                    pfor = ps.tile([128, 512], F32, tag="pb")
                    for k in range(4):
                        st, sp = (k == 0), (k == 3)
                        er = Ar[k][:, u0 : u0 + M]
                        ei = Ai[k][:, u0 : u0 + M]
                        orr = Ar[4 + k][:, u0 : u0 + M]
                        oi = Ai[4 + k][:, u0 : u0 + M]
                        if mu < 4:
                            nc.tensor.matmul(pfer[0:M], er, CeT[k][:], start=st, stop=False)
                            nc.tensor.matmul(pfer[0:M], ei, SenT[k][:], start=False, stop=sp,
                                             skip_group_check=True)
                            nc.tensor.matmul(pfor[0:M], orr, CoT[k][:], start=st, stop=False)
                            nc.tensor.matmul(pfor[0:M], oi, SonT[k][:], start=False, stop=sp,
                                             skip_group_check=True)
                        else:
                            nc.tensor.matmul(pfer[0:M], er, CeT[k][:], start=st, stop=sp)
                            nc.tensor.matmul(pfor[0:M], orr, CoT[k][:], start=st, stop=sp)
                    sc_ap = sw_t[0:M, 2 * mu : 2 * mu + 1]
                    feR = wpool.tile([128, 512], F32, tag="feR", bufs=1)
                    zrp = wpool.tile([128, 512], F32, tag="zrp", bufs=1)
                    zrm = wpool.tile([128, 512], F32, tag="zrm", bufs=1)
                    nc.scalar.activation(feR[0:M], pfer[0:M], AF.Copy, scale=sc_ap)
                    nc.vector.scalar_tensor_tensor(
                        zrp[0:M], pfor[0:M], sc_ap, feR[0:M], op0=AT.mult, op1=AT.add)
                    nc.vector.scalar_tensor_tensor(
                        zrm[0:M], pfor[0:M], sc_ap, feR[0:M], op0=AT.mult, op1=AT.subtract)
                    pnei = ps.tile([128, 512], F32, tag="pa")
                    pnoi = ps.tile([128, 512], F32, tag="pb")
                    for k in range(4):
                        st, sp = (k == 0), (k == 3)
                        er = Ar[k][:, u0 : u0 + M]
                        ei = Ai[k][:, u0 : u0 + M]
                        orr = Ar[4 + k][:, u0 : u0 + M]
                        oi = Ai[4 + k][:, u0 : u0 + M]
                        if mu < 4:
                            nc.tensor.matmul(pnei[0:M], ei, CeT[k][:], start=st, stop=False)
                            nc.tensor.matmul(pnei[0:M], er, SeT[k][:], start=False, stop=sp,
                                             skip_group_check=True)
                            nc.tensor.matmul(pnoi[0:M], oi, CoT[k][:], start=st, stop=False)
                            nc.tensor.matmul(pnoi[0:M], orr, SoT[k][:], start=False, stop=sp,
                                             skip_group_check=True)
                        else:
                            nc.tensor.matmul(pnei[0:M], er, SeT[k][:], start=st, stop=sp)
                            nc.tensor.matmul(pnoi[0:M], orr, SoT[k][:], start=st, stop=sp)
                    feI = wpool.tile([128, 512], F32, tag="feI", bufs=1)
                    zip_ = wpool.tile([128, 512], F32, tag="zip", bufs=1)
                    zim = wpool.tile([128, 512], F32, tag="zim", bufs=1)
                    nc.scalar.activation(feI[0:M], pnei[0:M], AF.Copy, scale=sc_ap)
                    nc.vector.scalar_tensor_tensor(
                        zip_[0:M], pnoi[0:M], sc_ap, feI[0:M], op0=AT.mult, op1=AT.add)
                    nc.vector.scalar_tensor_tensor(
                        zim[0:M], pnoi[0:M], sc_ap, feI[0:M], op0=AT.mult, op1=AT.subtract)

                    # U+/- = |sc*Z|^2 (scale already folded into Z);
                    # computed in place: up aliases zrp, um aliases zrm
                    up, um = zrp, zrm
                    nc.scalar.activation(up[0:M], zrp[0:M], AF.Square)
                    nc.scalar.activation(zip_[0:M], zip_[0:M], AF.Square)
                    nc.vector.tensor_tensor(out=up[0:M], in0=up[0:M], in1=zip_[0:M], op=AT.add)
                    nc.scalar.activation(zrm[0:M], zrm[0:M], AF.Square)
                    nc.scalar.activation(zim[0:M], zim[0:M], AF.Square)
                    nc.vector.tensor_tensor(out=um[0:M], in0=um[0:M], in1=zim[0:M], op=AT.add)
                    if mu == 0:
                        # zero DC: kills the catastrophic p2/q2 cancellation
                        nc.vector.tensor_copy(up[0:1, 0:1], zt[0:1, 0:1])
                    # fold to G strip: G[w']=U+(w')+U-(512-w')
                    nc.vector.tensor_tensor(
                        out=Gq[mu][0:M, 1:512], in0=up[0:M, 1:512],
                        in1=um[0:M, 511:0:-1], op=AT.add)
                    nc.vector.tensor_copy(Gq[mu][0:M, 0:1], up[0:M, 0:1])
                    nc.vector.tensor_copy(Gq[mu][0:M, 512:513], um[0:M, 0:1])
                    # p2 accumulation
                    rsp = wpool.tile([128, 1], F32, tag="rsp")
                    rsm = wpool.tile([128, 1], F32, tag="rsm")
                    nc.scalar.activation(up[0:M], up[0:M], AF.Square, accum_out=rsp[0:M])
                    nc.scalar.activation(um[0:M], um[0:M], AF.Square, accum_out=rsm[0:M])
                    nc.vector.scalar_tensor_tensor(
                        p2acc[0:M, 0:1], rsp[0:M], sw_t[0:M, 2 * mu + 1 : 2 * mu + 2],
                        p2acc[0:M, 0:1], op0=AT.mult, op1=AT.add)
                    nc.vector.scalar_tensor_tensor(
                        p2acc[0:M, 0:1], rsm[0:M], sw_t[0:M, 2 * mu + 1 : 2 * mu + 2],
                        p2acc[0:M, 0:1], op0=AT.mult, op1=AT.add)

            def dfrow(img, ci):
                """diagonal fold G + G^T (upper triangle) via PSUM accumulate,
                writing the bf16 barrel strips directly; one destination strip"""
                xb = Xb[img]
                if True:
                    for cj in range(ci, 4):
                        tp = ps.tile([128, 128], F32, tag="pd")
                        nc.tensor.matmul(tp[:], ident[:],
                                         Gq[ci][:, 128 * cj : 128 * cj + 128],
                                         start=True, stop=False)
                        nc.tensor.matmul(tp[:], Gq[cj][:, 128 * ci : 128 * ci + 128],
                                         ident[:], is_transpose=True,
                                         start=False, stop=True,
                                         skip_group_check=True)
                        if ci == cj:
                            nc.vector.tensor_tensor(
                                out=xb[ci][:, 128 * cj : 128 * cj + 128],
                                in0=tp[:], in1=mfd_t[:], op=AT.mult)
                        else:
                            nc.scalar.activation(
                                xb[ci][:, 128 * cj : 128 * cj + 128], tp[:], AF.Copy)
                    # column 512 += transpose of strip-4 block
                    tp4 = ps.tile([128, 128], F32, tag="pd")
                    nc.tensor.matmul(tp4[:], Gq[4][:, 128 * ci : 128 * ci + 128],
                                     ident[:], is_transpose=True,
                                     start=True, stop=False)
                    nc.tensor.matmul(tp4[:, 0:1], ident[:], Gq[ci][:, 512:513],
                                     start=False, stop=True, skip_group_check=True)
                    nc.scalar.activation(xb[ci][:, 512:513], tp4[:, 0:1], AF.Copy)

            def brlstrip(img, ci):
                """barrel shear: align each row's columns to radial bins"""
                xb = Xb[img]
                if True:
                    off = 0
                    for t in range(NROUNDS):
                        lo, hi, m = table[ci][t]
                        wdt = hi - lo
                        if wdt <= 0:
                            continue
                        bit = 1 << t
                        gps = ci >= 3 or (img == 1 and ci == 1)
                        eng = nc.gpsimd if gps else nc.vector
                        tmp = wpool.tile([128, 400 if gps else 512], BF16,
                                         tag=("btmpg" if gps else "btmp"), bufs=1)
                        eng.tensor_tensor(
                            out=tmp[:, 0:wdt], in0=xb[ci][:, lo:hi],
                            in1=bm_t[ci][:, off : off + wdt], op=AT.mult)
                        eng.tensor_tensor(
                            out=xb[ci][:, lo:hi], in0=xb[ci][:, lo:hi],
                            in1=tmp[:, 0:wdt], op=AT.subtract)
                        eng.tensor_tensor(
                            out=xb[ci][:, lo + bit : hi + bit],
                            in0=xb[ci][:, lo + bit : hi + bit],
                            in1=tmp[:, 0:wdt], op=AT.add)
                        off += wdt

            def red(img):
                """per-bin sums -> q2; loss = p2 - q2"""
                xb = Xb[img]
                ps_lo = ps.tile([1, 512], F32, tag="pa")
                ps_hi = ps.tile([1, 216], F32, tag="pb")
                for ci in range(5):
                    st, sp = (ci == 0), (ci == 4)
                    nc.tensor.matmul(ps_lo[:], onesb[:], xb[ci][:, 0:512], start=st, stop=sp)
                    nc.tensor.matmul(ps_hi[:], onesb[:], xb[ci][:, 512:WB], start=st, stop=sp)
                ssq = wpool.tile([1, WB], F32, tag="ssq", bufs=1)
                nc.scalar.activation(ssq[0:1, 0:512], ps_lo[:], AF.Square)
                nc.scalar.activation(ssq[0:1, 512:WB], ps_hi[:], AF.Square)
                nc.vector.tensor_tensor(out=ssq[:], in0=ssq[:], in1=ic_t[:], op=AT.mult)
                q2 = wpool.tile([1, 1], F32, tag="q2")
                nc.vector.tensor_reduce(q2[:], ssq[:], axis=mybir.AxisListType.X, op=AT.add)
                psp = ps.tile([1, 8], F32, tag="pd")
                nc.tensor.matmul(psp[:], ones[:], P2[img][:], start=True, stop=True)
                nc.vector.tensor_tensor(
                    out=lossv[0:1, img : img + 1], in0=psp[0:1, 0:1], in1=q2[:],
                    op=AT.subtract)

            def s2df(img):
                """s2 + fold + barrel, v-blocks descending so each strip's
                barrel overlaps the remaining blocks' matmuls"""
                s2pre(img)
                s2row(img, 4)
                # strip 4: the lone (512,512) diagonal cell keeps weight 1
                nc.scalar.activation(Xb[img][4][0:1, 512:513],
                                     Gq[4][0:1, 512:513], AF.Copy)
                brlstrip(img, 4)
                for mu in (3, 2, 1, 0):
                    s2row(img, mu)
                    dfrow(img, mu)
                    brlstrip(img, mu)

            # ---------------- program (software-pipelined over 2 images)
            # PE warmup: keep HAM un-throttled while the input/const DMAs
            # stream in (PE would otherwise idle ~20us and start cold)
            for i in range(96):
                pw = ps.tile([128, 128], F32, tag="pd")
                nc.tensor.matmul(pw[:], ident[:], ident[:], start=True, stop=True)

            for k in range(8):
                nc.sync.dma_start(Xt[k][:], x_p[0, 128 * k : 128 * k + 128, :])
            for k in range(8):
                nc.sync.dma_start(Cm_t[k][:], cm_p[128 * k : 128 * k + 128, :])
                nc.sync.dma_start(Sm_t[k][:], sm_p[128 * k : 128 * k + 128, :])
            nc.sync.dma_start(sw_t[:], sw_p[:])
            nc.sync.dma_start(ic_t[:], ic_p[:])
            for k in range(4):
                sl = slice(128 * k, 128 * k + 128)
                nc.sync.dma_start(CeT[k][:], ce_p[sl, :])
                nc.sync.dma_start(SeT[k][:], se_p[sl, :])
                nc.sync.dma_start(SenT[k][:], sen_p[sl, :])
                nc.sync.dma_start(CoT[k][:], co_p[sl, :])
                nc.sync.dma_start(SoT[k][:], so_p[sl, :])
                nc.sync.dma_start(SonT[k][:], son_p[sl, :])

            sc = nc.named_scope("s1_0"); sc.__enter__()
            s1(0)
            sc.__exit__(None, None, None)
            # prefetch image 1 and the barrel masks behind the s1 constants
            for k in range(8):
                nc.sync.dma_start(Xt[k][:], x_p[1, 128 * k : 128 * k + 128, :])
            for ci in range(5):
                wci = sum(hi - lo for (lo, hi, m) in table[ci])
                if wci > 0:
                    nc.sync.dma_start(bm_t[ci][:, 0:wci],
                                      bm_p[128 * ci : 128 * ci + 128, 0:wci])
            nc.sync.dma_start(mfd_t[:], md_p[:])

            sc = nc.named_scope("s2_0"); sc.__enter__()
            s2df(0)
            sc.__exit__(None, None, None)
            sc = nc.named_scope("s1_1"); sc.__enter__()
            s1(1)
            sc.__exit__(None, None, None)
            sc = nc.named_scope("red_0"); sc.__enter__()
            red(0)
            sc.__exit__(None, None, None)
            sc = nc.named_scope("s2_1"); sc.__enter__()
            s2df(1)
            sc.__exit__(None, None, None)
            sc = nc.named_scope("red_1"); sc.__enter__()
            red(1)
            sc.__exit__(None, None, None)

            nc.sync.dma_start(out_p[:], lossv[:])

    nc.compile()
    return nc


def _get_nc():
    if "nc" not in _CACHE:
        _CACHE["nc"] = _build_nc()
    return _CACHE["nc"]


# ---------------------------------------------------------------- entry point
def kernel(prob_cg: np.ndarray) -> np.ndarray:
    hc = _host_constants()
    nc = _get_nc()
    x = prob_cg[:, 0, :, :].astype(np.float32)
    # pre-permute columns to [even m | odd m] so s1 writes even/odd A blocks
    xp = np.ascontiguousarray(
        np.concatenate([x[:, :, 0::2], x[:, :, 1::2]], axis=2))
    in_maps = []
    for i in range(N_CORES):
        in_maps.append(
            dict(
                x=xp[2 * i : 2 * i + 2],
                cm=hc["Cm"], sm=hc["Sm"],
                ce=hc["Ce"], se=hc["Se"], sen=hc["Sen"],
                co=hc["Co"], so=hc["So"], son=hc["Son"],
                bm=hc["bmask_bf16"], md=hc["mfd_bf16"],
                sw=hc["swc"], ic=hc["invc"],
            )
        )
    trace = os.environ.get("AT_TRACE", "0") == "1"
    kw = {}
    if trace and os.environ.get("AT_TMPDIR"):
        kw["tmpdir"] = os.environ["AT_TMPDIR"]
    res = run_bass_kernel_spmd(nc, in_maps, core_ids=list(range(N_CORES)), trace=trace, **kw)
    if trace and res.exec_time_ns is not None:
        print(f"HW exec time: {res.exec_time_ns} ns")
        if res.per_core_scope_times:
            for kname, v in sorted(res.per_core_scope_times.items()):
                print(f"  scope {kname}: {v}")
    losses = np.concatenate([r["out"].reshape(-1) for r in res.results])
    loss = losses.mean() + (H * H) * (EPS * EPS)
    return np.float32(WA * loss)


# revision 35
# speedup vs baseline: 1.0305x; 1.0169x over previous
"""AnisotropySuppressionLoss on 8 TRN2 NeuronCores (Bass/Tile), v2.

Per image (1024x1024 fp32):
  s1: A[m, v] = DFT_u x[u, m], v = 0..512 (real-input half spectrum),
      via f32r matmuls. Host pre-permutes image columns to [even m | odd m]
      so A lands in even/odd m' blocks.
  s2: radix-2 over m with the twiddle absorbed into the odd-m DFT matrix:
      Fe = DFT_{even m}(A), Fo' = sum_{odd m} A e^{-2pi i m w/1024};
      Z(w) = Fe + Fo', Z(w+512) = Fe - Fo' (w = 0..511). Halves matmul work
      vs a direct 1024-wide second DFT; butterflies are 4 DVE adds/block.
  power: U+/- = w_v/H^2 * |Z|^2 via ACT squares (scale) + DVE stt squares.
  fold:  G[v,w'] = U+(w') + U-(512-w') into bf16 strips; diagonal fold
      G + G^T via PE transpose accumulated in PSUM (ACT copy back).
  radial: barrel-shift shear (exact), bf16, 8 rounds/strip.
  loss_img = sum w*P^2/w - sum_b S_b^2/c_b  (+ H*W*eps^2 on host).
Data-parallel: batch 16 -> 2 images/core on 8 cores; host averages.
"""

import os
import sys

sys.path.insert(0, "/opt/trn_rl_repo")

import numpy as np

import concourse.bass as bass
import concourse.tile as tile
from concourse import bacc, mybir
from concourse.bass_utils import run_bass_kernel_spmd
from concourse.masks import make_identity

F32 = mybir.dt.float32
F32R = mybir.dt.float32r
BF16 = mybir.dt.bfloat16

H = 1024
NQ = 513          # quadrant size (|du|, |dv| in 0..512)
NB = 725          # radial bins 0..724
WB = 728          # barrel buffer width
NROUNDS = 8
N_CORES = 8
IMGS_PER_CORE = 2
WA = 0.002
EPS = 1e-12
CHUNKS = [(0, 128), (128, 256), (256, 384), (384, 512), (512, 513)]
AF = mybir.ActivationFunctionType

_CACHE = {}


# ---------------------------------------------------------------- host consts
def _gen_barrel_masks():
    """Per (chunk, round): (lo, hi, move_mask[128, hi-lo]) in quadrant coords.
    Cells (a, b>=a) carry delta = bin - b; each round moves cells with bit t
    set right by 2^t. Merges are exact (same remaining delta)."""
    rem = -np.ones((NQ, WB), dtype=np.int64)
    for a in range(NQ):
        cols = np.arange(a, NQ)
        bins = np.floor(np.sqrt(a * a + cols.astype(np.float64) ** 2)).astype(np.int64)
        rem[a, cols] = bins - cols
    table = [[] for _ in CHUNKS]
    for t in range(NROUNDS):
        bit = 1 << t
        move = (rem >= 0) & ((rem & bit) != 0)
        for ci, (c0, c1) in enumerate(CHUNKS):
            mv = move[c0:c1]
            cols_any = np.nonzero(mv.any(axis=0))[0]
            if len(cols_any) == 0:
                table[ci].append((0, 0, None))
            else:
                # widen span to even bounds: 4B-aligned bf16 slices let the
                # DVE pick its 2x packed mode
                lo, hi = int(cols_any[0]) & ~1, int(cols_any[-1]) + 1
                hi += hi & 1
                m = np.zeros((128, hi - lo), dtype=np.float32)
                m[: c1 - c0] = mv[:, lo:hi]
                table[ci].append((lo, hi, m))
        new_rem = -np.ones_like(rem)
        stay = (rem >= 0) & ~move
        new_rem[stay] = rem[stay]
        sr, sc = np.nonzero(move)
        dc = sc + bit
        landing = rem[sr, sc] - bit
        cur = new_rem[sr, dc]
        assert ((cur == -1) | (cur == landing)).all()
        new_rem[sr, dc] = landing
        rem = new_rem
    assert (rem[rem >= 0] == 0).all()
    return table


def _host_constants():
    if "consts" in _CACHE:
        return _CACHE["consts"]
    import ml_dtypes

    u = np.arange(H, dtype=np.float64)
    v = np.arange(520, dtype=np.float64)
    ang1 = 2.0 * np.pi * np.outer(u, v) / H
    Cm = np.cos(ang1).astype(np.float32)        # [1024, 520]
    Sm = np.sin(ang1).astype(np.float32)

    mp = np.arange(512, dtype=np.float64)
    w = np.arange(512, dtype=np.float64)
    ae = 2.0 * np.pi * np.outer(mp, w) / 512.0
    ao = 2.0 * np.pi * np.outer(2 * mp + 1, w) / 1024.0
    Ce = np.cos(ae).astype(np.float32)
    Se = np.sin(ae).astype(np.float32)
    Co = np.cos(ao).astype(np.float32)
    So = np.sin(ao).astype(np.float32)

    # radial bin counts exactly as reference._radial_bins (unshifted coords)
    y = np.minimum(np.arange(H), H - np.arange(H))
    yy, xx = np.meshgrid(y, y, indexing="ij")
    dist = np.sqrt((xx.astype(np.float64)) ** 2 + yy.astype(np.float64) ** 2)
    bins_full = np.clip(dist.astype(np.int32), 0, NB - 1)
    counts = np.bincount(bins_full.reshape(-1), minlength=NB).astype(np.float64)
    invc = np.zeros((1, WB), dtype=np.float32)
    invc[0, :NB] = (1.0 / counts).astype(np.float32)

    # row weights w_v for v = 0..512; sw cols: 2*mu = sqrt(w)/H (ACT square
    # scale), 2*mu+1 = 1/w (p2 accumulation), 10+mu = w/H^2 (DVE stt square)
    wv = np.full(NQ, 2.0)
    wv[0] = 1.0
    wv[512] = 1.0
    swc = np.zeros((128, 16), dtype=np.float32)
    for mu in range(5):
        c0, c1 = CHUNKS[mu]
        n = c1 - c0
        swc[:n, 2 * mu] = (np.sqrt(wv[c0:c1]) / H).astype(np.float32)
        swc[:n, 2 * mu + 1] = (1.0 / wv[c0:c1]).astype(np.float32)
        swc[:n, 10 + mu] = (wv[c0:c1] / (H * H)).astype(np.float32)

    table = _gen_barrel_masks()
    chunk_w = [max(1, sum(hi - lo for (lo, hi, m) in table[ci])) for ci in range(5)]
    maxw = max(chunk_w)
    bmask = np.zeros((640, maxw), dtype=np.float32)
    for ci in range(5):
        off = 0
        for (lo, hi, m) in table[ci]:
            if m is None:
                continue
            bmask[128 * ci : 128 * ci + 128, off : off + hi - lo] = m
            off += hi - lo

    # diagonal-block mask: 0 below diag, 0.5 on diag, 1 above (block-local)
    a = np.arange(128)
    mfd = (a[None, :] > a[:, None]).astype(np.float32)
    mfd[a, a] = 0.5

    _CACHE["consts"] = dict(
        Cm=Cm, Sm=Sm, Ce=Ce, Se=Se, Sen=(-Se), Co=Co, So=So, Son=(-So),
        invc=invc, swc=swc,
        bmask_bf16=bmask.astype(ml_dtypes.bfloat16),
        mfd_bf16=mfd.astype(ml_dtypes.bfloat16),
        table=table, maxw=maxw, counts=counts,
    )
    return _CACHE["consts"]


# ---------------------------------------------------------------- device build
def _build_nc():
    hc = _host_constants()
    table, maxw = hc["table"], hc["maxw"]

    nc = bacc.Bacc("TRN2", target_bir_lowering=False, debug=False)
    x_p = nc.declare_dram_parameter("x", [IMGS_PER_CORE, H, H], F32R, isOutput=False)
    cm_p = nc.declare_dram_parameter("cm", [H, 520], F32R, isOutput=False)
    sm_p = nc.declare_dram_parameter("sm", [H, 520], F32R, isOutput=False)
    ce_p = nc.declare_dram_parameter("ce", [512, 512], F32R, isOutput=False)
    se_p = nc.declare_dram_parameter("se", [512, 512], F32R, isOutput=False)
    sen_p = nc.declare_dram_parameter("sen", [512, 512], F32R, isOutput=False)
    co_p = nc.declare_dram_parameter("co", [512, 512], F32R, isOutput=False)
    so_p = nc.declare_dram_parameter("so", [512, 512], F32R, isOutput=False)
    son_p = nc.declare_dram_parameter("son", [512, 512], F32R, isOutput=False)
    bm_p = nc.declare_dram_parameter("bm", [640, maxw], BF16, isOutput=False)
    md_p = nc.declare_dram_parameter("md", [128, 128], BF16, isOutput=False)
    sw_p = nc.declare_dram_parameter("sw", [128, 16], F32, isOutput=False)
    ic_p = nc.declare_dram_parameter("ic", [1, WB], F32, isOutput=False)
    out_p = nc.declare_dram_parameter("out", [1, IMGS_PER_CORE], F32, isOutput=True)

    AT = mybir.AluOpType

    with tile.TileContext(nc) as tc:
        with (
            tc.tile_pool(name="const", bufs=1) as cpool,
            tc.tile_pool(name="xin", bufs=1) as xpool,
            tc.tile_pool(name="arr", bufs=1) as apool,
            tc.tile_pool(name="quad", bufs=1) as qpool,
            tc.tile_pool(name="work", bufs=2) as wpool,
            tc.tile_pool(name="ps", bufs=2, space="PSUM") as ps,
        ):
            # ---------------- constants
            Cm_t = [cpool.tile([128, 520], F32R, tag=f"cm{k}", name=f"cm{k}") for k in range(8)]
            Sm_t = [cpool.tile([128, 520], F32R, tag=f"sm{k}", name=f"sm{k}") for k in range(8)]
            CeT = [cpool.tile([128, 512], F32R, tag=f"ce{k}", name=f"ce{k}") for k in range(4)]
            SeT = [cpool.tile([128, 512], F32R, tag=f"sE{k}", name=f"sE{k}") for k in range(4)]
            SenT = [cpool.tile([128, 512], F32R, tag=f"sn{k}", name=f"sn{k}") for k in range(4)]
            CoT = [cpool.tile([128, 512], F32R, tag=f"co{k}", name=f"co{k}") for k in range(4)]
            SoT = [cpool.tile([128, 512], F32R, tag=f"sO{k}", name=f"sO{k}") for k in range(4)]
            SonT = [cpool.tile([128, 512], F32R, tag=f"sm{k}b", name=f"sm{k}b") for k in range(4)]
            bm_t = [
                cpool.tile([128, max(1, sum(hi - lo for (lo, hi, m) in table[ci]))],
                           BF16, tag=f"bm{ci}", name=f"bm{ci}")
                for ci in range(5)
            ]
            mfd_t = cpool.tile([128, 128], BF16, tag="mfd")
            sw_t = cpool.tile([128, 16], F32, tag="sw")
            ic_t = cpool.tile([1, WB], F32, tag="ic")
            ident = cpool.tile([128, 128], F32, tag="ident")
            make_identity(nc, ident[:])
            identr = cpool.tile([128, 128], F32R, tag="identr")
            nc.vector.tensor_copy(ident[:], ident[:])
            ones32 = cpool.tile([128, 1], F32, tag="ones32")
            nc.gpsimd.memset(ones32[:], 1.0)
            ones = cpool.tile([128, 1], F32R, tag="ones")
            nc.vector.tensor_copy(ones[:], ones32[:])
            onesb = cpool.tile([128, 1], BF16, tag="onesb")
            nc.vector.tensor_copy(onesb[:], ones32[:])
            zt = cpool.tile([128, 8], F32, tag="zt")
            nc.gpsimd.memset(zt[:], 0.0)
            lossv = cpool.tile([1, IMGS_PER_CORE], F32, tag="lossv")

            # ---------------- per-image persistent arrays
            Xt = [xpool.tile([128, H], F32R, tag=f"x{k}", name=f"x{k}") for k in range(8)]
            # A blocks: j=0..3 even m' chunks, j=4..7 odd m' chunks
            Ar = [apool.tile([128, NQ], F32R, tag=f"ar{j}", name=f"ar{j}") for j in range(8)]
            Ai = [apool.tile([128, NQ], F32R, tag=f"ai{j}", name=f"ai{j}") for j in range(8)]
            # f32r folded-G strips (shared across images) + bf16 barrel strips
            # double-buffered across images (for the pipelined red)
            Gq = [qpool.tile([128, NQ], F32, tag=f"gq{ci}", name=f"gq{ci}")
                  for ci in range(5)]
            Xb = [[qpool.tile([128, WB], BF16, tag=f"xb{p}_{ci}", name=f"xb{p}_{ci}")
                   for ci in range(5)] for p in range(IMGS_PER_CORE)]
            P2 = [qpool.tile([128, 8], F32R, tag=f"p2acc{p}", name=f"p2acc{p}")
                  for p in range(IMGS_PER_CORE)]

            def s1(img):
                """first DFT: fills Ar/Ai blocks; Ai = -Im(A)"""
                for m in range(8):
                    pr_lo = ps.tile([128, 512], F32, tag="pa")
                    pr_hi = ps.tile([128, 8], F32, tag="pd")
                    pt_lo = ps.tile([128, 512], F32, tag="pb")
                    for k in range(8):
                        lhs = Xt[k][:, 128 * m : 128 * m + 128]
                        st, sp = (k == 0), (k == 7)
                        nc.tensor.matmul(pr_lo[:], lhs, Cm_t[k][:, 0:512], start=st, stop=sp)
                        nc.tensor.matmul(pr_hi[:], lhs, Cm_t[k][:, 512:520], start=st, stop=sp)
                        nc.tensor.matmul(pt_lo[:], lhs, Sm_t[k][:, 0:512], start=st, stop=sp)
                    nc.scalar.activation(Ar[m][:, 0:512], pr_lo[:], AF.Copy)
                    nc.scalar.activation(Ar[m][:, 512:513], pr_hi[:, 0:1], AF.Copy)
                    nc.scalar.activation(Ai[m][:, 0:512], pt_lo[:], AF.Copy)
                    nc.vector.tensor_copy(Ai[m][:, 512:513], zt[:, 0:1])

            def s2pre(img):
                """zero barrel strips, the strip-4 G row, and p2acc"""
                xb = Xb[img]
                for ci in range(5):
                    nc.gpsimd.memset(xb[ci][:], 0.0)
                nc.gpsimd.memset(Gq[4][:], 0.0)
                nc.vector.tensor_copy(P2[img][:], zt[:])

            def s2row(img, mu):
                """second DFT (even/odd split) + power + fold, one v-block"""
                p2acc = P2[img]
                if True:
                    M = 128 if mu < 4 else 1
                    u0 = 128 * mu
                    pfer = ps.tile([128, 512], F32, tag="pa")
                    pfor = ps.tile([128, 512], F32, tag="pb")
                    for k in range(4):
                        st, sp = (k == 0), (k == 3)
                        er = Ar[k][:, u0 : u0 + M]
                        ei = Ai[k][:, u0 : u0 + M]
                        orr = Ar[4 + k][:, u0 : u0 + M]
                        oi = Ai[4 + k][:, u0 : u0 + M]
                        if mu < 4:
                            nc.tensor.matmul(pfer[0:M], er, CeT[k][:], start=st, stop=False)
                            nc.tensor.matmul(pfer[0:M], ei, SenT[k][:], start=False, stop=sp,
                                             skip_group_check=True)
                            nc.tensor.matmul(pfor[0:M], orr, CoT[k][:], start=st, stop=False)
                            nc.tensor.matmul(pfor[0:M], oi, SonT[k][:], start=False, stop=sp,
                                             skip_group_check=True)
                        else:
                            nc.tensor.matmul(pfer[0:M], er, CeT[k][:], start=st, stop=sp)
                            nc.tensor.matmul(pfor[0:M], orr, CoT[k][:], start=st, stop=sp)
                    sc_ap = sw_t[0:M, 2 * mu : 2 * mu + 1]
                    feR = wpool.tile([128, 512], F32, tag="feR", bufs=1)
                    zrp = wpool.tile([128, 512], F32, tag="zrp", bufs=1)
                    zrm = wpool.tile([128, 512], F32, tag="zrm", bufs=1)
                    nc.scalar.activation(feR[0:M], pfer[0:M], AF.Copy, scale=sc_ap)
                    nc.vector.scalar_tensor_tensor(
                        zrp[0:M], pfor[0:M], sc_ap, feR[0:M], op0=AT.mult, op1=AT.add)
                    nc.vector.scalar_tensor_tensor(
                        zrm[0:M], pfor[0:M], sc_ap, feR[0:M], op0=AT.mult, op1=AT.subtract)
                    pnei = ps.tile([128, 512], F32, tag="pa")
                    pnoi = ps.tile([128, 512], F32, tag="pb")
                    for k in range(4):
                        st, sp = (k == 0), (k == 3)
                        er = Ar[k][:, u0 : u0 + M]
                        ei = Ai[k][:, u0 : u0 + M]
                        orr = Ar[4 + k][:, u0 : u0 + M]
                        oi = Ai[4 + k][:, u0 : u0 + M]
                        if mu < 4:
                            nc.tensor.matmul(pnei[0:M], ei, CeT[k][:], start=st, stop=False)
                            nc.tensor.matmul(pnei[0:M], er, SeT[k][:], start=False, stop=sp,
                                             skip_group_check=True)
                            nc.tensor.matmul(pnoi[0:M], oi, CoT[k][:], start=st, stop=False)
                            nc.tensor.matmul(pnoi[0:M], orr, SoT[k][:], start=False, stop=sp,
                                             skip_group_check=True)
                        else:
                            nc.tensor.matmul(pnei[0:M], er, SeT[k][:], start=st, stop=sp)
                            nc.tensor.matmul(pnoi[0:M], orr, SoT[k][:], start=st, stop=sp)
                    feI = wpool.tile([128, 512], F32, tag="feI", bufs=1)
                    zip_ = wpool.tile([128, 512], F32, tag="zip", bufs=1)
                    zim = wpool.tile([128, 512], F32, tag="zim", bufs=1)
                    nc.scalar.activation(feI[0:M], pnei[0:M], AF.Copy, scale=sc_ap)
                    nc.vector.scalar_tensor_tensor(
                        zip_[0:M], pnoi[0:M], sc_ap, feI[0:M], op0=AT.mult, op1=AT.add)
                    nc.vector.scalar_tensor_tensor(
                        zim[0:M], pnoi[0:M], sc_ap, feI[0:M], op0=AT.mult, op1=AT.subtract)

                    # U+/- = |sc*Z|^2 (scale already folded into Z);
                    # computed in place: up aliases zrp, um aliases zrm
                    up, um = zrp, zrm
                    nc.scalar.activation(up[0:M], zrp[0:M], AF.Square)
                    nc.scalar.activation(zip_[0:M], zip_[0:M], AF.Square)
                    nc.vector.tensor_tensor(out=up[0:M], in0=up[0:M], in1=zip_[0:M], op=AT.add)
                    nc.scalar.activation(zrm[0:M], zrm[0:M], AF.Square)
                    nc.scalar.activation(zim[0:M], zim[0:M], AF.Square)
                    nc.vector.tensor_tensor(out=um[0:M], in0=um[0:M], in1=zim[0:M], op=AT.add)
                    if mu == 0:
                        # zero DC: kills the catastrophic p2/q2 cancellation
                        nc.vector.tensor_copy(up[0:1, 0:1], zt[0:1, 0:1])
                    # fold to G strip: G[w']=U+(w')+U-(512-w')
                    nc.vector.tensor_tensor(
                        out=Gq[mu][0:M, 1:512], in0=up[0:M, 1:512],
                        in1=um[0:M, 511:0:-1], op=AT.add)
                    nc.vector.tensor_copy(Gq[mu][0:M, 0:1], up[0:M, 0:1])
                    nc.vector.tensor_copy(Gq[mu][0:M, 512:513], um[0:M, 0:1])
                    # p2 accumulation
                    rsp = wpool.tile([128, 1], F32, tag="rsp")
                    rsm = wpool.tile([128, 1], F32, tag="rsm")
                    nc.scalar.activation(up[0:M], up[0:M], AF.Square, accum_out=rsp[0:M])
                    nc.scalar.activation(um[0:M], um[0:M], AF.Square, accum_out=rsm[0:M])
                    nc.vector.scalar_tensor_tensor(
                        p2acc[0:M, 0:1], rsp[0:M], sw_t[0:M, 2 * mu + 1 : 2 * mu + 2],
                        p2acc[0:M, 0:1], op0=AT.mult, op1=AT.add)
                    nc.vector.scalar_tensor_tensor(
                        p2acc[0:M, 0:1], rsm[0:M], sw_t[0:M, 2 * mu + 1 : 2 * mu + 2],
                        p2acc[0:M, 0:1], op0=AT.mult, op1=AT.add)

            def dfrow(img, ci):
                """diagonal fold G + G^T (upper triangle) via PSUM accumulate,
                writing the bf16 barrel strips directly; one destination strip"""
                xb = Xb[img]
                if True:
                    for cj in range(ci, 4):
                        tp = ps.tile([128, 128], F32, tag="pd")
                        if ci == cj:
                            nc.tensor.matmul(tp[:], ident[:],
                                             Gq[ci][:, 128 * cj : 128 * cj + 128],
                                             start=True, stop=False)
                            nc.tensor.matmul(tp[:], Gq[cj][:, 128 * ci : 128 * ci + 128],
                                             ident[:], is_transpose=True,
                                             start=False, stop=True,
                                             skip_group_check=True)
                            nc.vector.tensor_tensor(
                                out=xb[ci][:, 128 * cj : 128 * cj + 128],
                                in0=tp[:], in1=mfd_t[:], op=AT.mult)
                        else:
                            nc.tensor.matmul(tp[:], Gq[cj][:, 128 * ci : 128 * ci + 128],
                                             ident[:], is_transpose=True,
                                             start=True, stop=True)
                            nc.vector.tensor_tensor(
                                out=xb[ci][:, 128 * cj : 128 * cj + 128],
                                in0=tp[:], in1=Gq[ci][:, 128 * cj : 128 * cj + 128],
                                op=AT.add)
                    # column 512 += transpose of strip-4 block
                    tp4 = ps.tile([128, 128], F32, tag="pd")
                    nc.tensor.matmul(tp4[:], Gq[4][:, 128 * ci : 128 * ci + 128],
                                     ident[:], is_transpose=True,
                                     start=True, stop=True)
                    nc.vector.tensor_tensor(
                        out=xb[ci][:, 512:513], in0=tp4[:, 0:1],
                        in1=Gq[ci][:, 512:513], op=AT.add)

            def brlstrip(img, ci):
                """barrel shear: align each row's columns to radial bins"""
                xb = Xb[img]
                if True:
                    off = 0
                    for t in range(NROUNDS):
                        lo, hi, m = table[ci][t]
                        wdt = hi - lo
                        if wdt <= 0:
                            continue
                        bit = 1 << t
                        gps = ci >= 3
                        eng = nc.gpsimd if gps else nc.vector
                        tmp = wpool.tile([128, 192 if gps else 512], BF16,
                                         tag=("btmpg" if gps else "btmp"), bufs=1)
                        eng.tensor_tensor(
                            out=tmp[:, 0:wdt], in0=xb[ci][:, lo:hi],
                            in1=bm_t[ci][:, off : off + wdt], op=AT.mult)
                        eng.tensor_tensor(
                            out=xb[ci][:, lo:hi], in0=xb[ci][:, lo:hi],
                            in1=tmp[:, 0:wdt], op=AT.subtract)
                        eng.tensor_tensor(
                            out=xb[ci][:, lo + bit : hi + bit],
                            in0=xb[ci][:, lo + bit : hi + bit],
                            in1=tmp[:, 0:wdt], op=AT.add)
                        off += wdt

            def red(img):
                """per-bin sums -> q2; loss = p2 - q2"""
                xb = Xb[img]
                ps_lo = ps.tile([1, 512], F32, tag="pa")
                ps_hi = ps.tile([1, 216], F32, tag="pb")
                for ci in range(5):
                    st, sp = (ci == 0), (ci == 4)
                    nc.tensor.matmul(ps_lo[:], onesb[:], xb[ci][:, 0:512], start=st, stop=sp)
                    nc.tensor.matmul(ps_hi[:], onesb[:], xb[ci][:, 512:WB], start=st, stop=sp)
                ssq = wpool.tile([1, WB], F32, tag="ssq", bufs=1)
                nc.scalar.activation(ssq[0:1, 0:512], ps_lo[:], AF.Square)
                nc.scalar.activation(ssq[0:1, 512:WB], ps_hi[:], AF.Square)
                nc.vector.tensor_tensor(out=ssq[:], in0=ssq[:], in1=ic_t[:], op=AT.mult)
                q2 = wpool.tile([1, 1], F32, tag="q2")
                nc.vector.tensor_reduce(q2[:], ssq[:], axis=mybir.AxisListType.X, op=AT.add)
                psp = ps.tile([1, 8], F32, tag="pd")
                nc.tensor.matmul(psp[:], ones[:], P2[img][:], start=True, stop=True)
                nc.vector.tensor_tensor(
                    out=lossv[0:1, img : img + 1], in0=psp[0:1, 0:1], in1=q2[:],
                    op=AT.subtract)

            def s2df(img):
                """s2 + fold + barrel, v-blocks descending so each strip's
                barrel overlaps the remaining blocks' matmuls"""
                s2pre(img)
                s2row(img, 4)
                # strip 4: the lone (512,512) diagonal cell keeps weight 1
                nc.scalar.activation(Xb[img][4][0:1, 512:513],
                                     Gq[4][0:1, 512:513], AF.Copy)
                brlstrip(img, 4)
                for mu in (3, 2, 1, 0):
                    s2row(img, mu)
                    dfrow(img, mu)
                    brlstrip(img, mu)

            # ---------------- program (software-pipelined over 2 images)
            # PE warmup: keep HAM un-throttled while the input/const DMAs
            # stream in (PE would otherwise idle ~20us and start cold)
            for i in range(96):
                pw = ps.tile([128, 128], F32, tag="pd")
                nc.tensor.matmul(pw[:], ident[:], ident[:], start=True, stop=True)

            for k in range(8):
                nc.sync.dma_start(Xt[k][:], x_p[0, 128 * k : 128 * k + 128, :])
            for k in range(8):
                nc.sync.dma_start(Cm_t[k][:], cm_p[128 * k : 128 * k + 128, :])
                nc.sync.dma_start(Sm_t[k][:], sm_p[128 * k : 128 * k + 128, :])
            nc.sync.dma_start(sw_t[:], sw_p[:])
            nc.sync.dma_start(ic_t[:], ic_p[:])
            for k in range(4):
                sl = slice(128 * k, 128 * k + 128)
                nc.sync.dma_start(CeT[k][:], ce_p[sl, :])
                nc.sync.dma_start(SeT[k][:], se_p[sl, :])
                nc.sync.dma_start(SenT[k][:], sen_p[sl, :])
                nc.sync.dma_start(CoT[k][:], co_p[sl, :])
                nc.sync.dma_start(SoT[k][:], so_p[sl, :])
                nc.sync.dma_start(SonT[k][:], son_p[sl, :])

            sc = nc.named_scope("s1_0"); sc.__enter__()
            s1(0)
            sc.__exit__(None, None, None)
            # prefetch image 1 and the barrel masks behind the s1 constants
            for k in range(8):
                nc.sync.dma_start(Xt[k][:], x_p[1, 128 * k : 128 * k + 128, :])
            for ci in range(5):
                wci = sum(hi - lo for (lo, hi, m) in table[ci])
                if wci > 0:
                    nc.sync.dma_start(bm_t[ci][:, 0:wci],
                                      bm_p[128 * ci : 128 * ci + 128, 0:wci])
            nc.sync.dma_start(mfd_t[:], md_p[:])

            sc = nc.named_scope("s2_0"); sc.__enter__()
            s2df(0)
            sc.__exit__(None, None, None)
            sc = nc.named_scope("s1_1"); sc.__enter__()
            s1(1)
            sc.__exit__(None, None, None)
            sc = nc.named_scope("red_0"); sc.__enter__()
            red(0)
            sc.__exit__(None, None, None)
            sc = nc.named_scope("s2_1"); sc.__enter__()
            s2df(1)
            sc.__exit__(None, None, None)
            sc = nc.named_scope("red_1"); sc.__enter__()
            red(1)
            sc.__exit__(None, None, None)

            nc.sync.dma_start(out_p[:], lossv[:])

    nc.compile()
    return nc


def _get_nc():
    if "nc" not in _CACHE:
        _CACHE["nc"] = _build_nc()
    return _CACHE["nc"]


# ---------------------------------------------------------------- entry point
def kernel(prob_cg: np.ndarray) -> np.ndarray:
    hc = _host_constants()
    nc = _get_nc()
    x = prob_cg[:, 0, :, :].astype(np.float32)
    # pre-permute columns to [even m | odd m] so s1 writes even/odd A blocks
    xp = np.ascontiguousarray(
        np.concatenate([x[:, :, 0::2], x[:, :, 1::2]], axis=2))
    in_maps = []
    for i in range(N_CORES):
        in_maps.append(
            dict(
                x=xp[2 * i : 2 * i + 2],
                cm=hc["Cm"], sm=hc["Sm"],
                ce=hc["Ce"], se=hc["Se"], sen=hc["Sen"],
                co=hc["Co"], so=hc["So"], son=hc["Son"],
                bm=hc["bmask_bf16"], md=hc["mfd_bf16"],
                sw=hc["swc"], ic=hc["invc"],
            )
        )
    trace = os.environ.get("AT_TRACE", "0") == "1"
    kw = {}
    if trace and os.environ.get("AT_TMPDIR"):
        kw["tmpdir"] = os.environ["AT_TMPDIR"]
    res = run_bass_kernel_spmd(nc, in_maps, core_ids=list(range(N_CORES)), trace=trace, **kw)
    if trace and res.exec_time_ns is not None:
        print(f"HW exec time: {res.exec_time_ns} ns")
        if res.per_core_scope_times:
            for kname, v in sorted(res.per_core_scope_times.items()):
                print(f"  scope {kname}: {v}")
    losses = np.concatenate([r["out"].reshape(-1) for r in res.results])
    loss = losses.mean() + (H * H) * (EPS * EPS)
    return np.float32(WA * loss)


# revision 36
# speedup vs baseline: 1.0313x; 1.0008x over previous
"""AnisotropySuppressionLoss on 8 TRN2 NeuronCores (Bass/Tile), v2.

Per image (1024x1024 fp32):
  s1: A[m, v] = DFT_u x[u, m], v = 0..512 (real-input half spectrum),
      via f32r matmuls. Host pre-permutes image columns to [even m | odd m]
      so A lands in even/odd m' blocks.
  s2: radix-2 over m with the twiddle absorbed into the odd-m DFT matrix:
      Fe = DFT_{even m}(A), Fo' = sum_{odd m} A e^{-2pi i m w/1024};
      Z(w) = Fe + Fo', Z(w+512) = Fe - Fo' (w = 0..511). Halves matmul work
      vs a direct 1024-wide second DFT; butterflies are 4 DVE adds/block.
  power: U+/- = w_v/H^2 * |Z|^2 via ACT squares (scale) + DVE stt squares.
  fold:  G[v,w'] = U+(w') + U-(512-w') into bf16 strips; diagonal fold
      G + G^T via PE transpose accumulated in PSUM (ACT copy back).
  radial: barrel-shift shear (exact), bf16; strips 0-2 on DVE,
      3-4 on GpSimd; PE warmup stream covers the initial DMA window.
  loss_img = sum w*P^2/w - sum_b S_b^2/c_b  (+ H*W*eps^2 on host).
Data-parallel: batch 16 -> 2 images/core on 8 cores; host averages.
"""

import os
import sys

sys.path.insert(0, "/opt/trn_rl_repo")

import numpy as np

import concourse.bass as bass
import concourse.tile as tile
from concourse import bacc, mybir
from concourse.bass_utils import run_bass_kernel_spmd
from concourse.masks import make_identity

F32 = mybir.dt.float32
F32R = mybir.dt.float32r
BF16 = mybir.dt.bfloat16

H = 1024
NQ = 513          # quadrant size (|du|, |dv| in 0..512)
NB = 725          # radial bins 0..724
WB = 728          # barrel buffer width
NROUNDS = 8
N_CORES = 8
IMGS_PER_CORE = 2
WA = 0.002
EPS = 1e-12
CHUNKS = [(0, 128), (128, 256), (256, 384), (384, 512), (512, 513)]
AF = mybir.ActivationFunctionType

_CACHE = {}


# ---------------------------------------------------------------- host consts
def _gen_barrel_masks():
    """Per (chunk, round): (lo, hi, move_mask[128, hi-lo]) in quadrant coords.
    Cells (a, b>=a) carry delta = bin - b; each round moves cells with bit t
    set right by 2^t. Merges are exact (same remaining delta)."""
    rem = -np.ones((NQ, WB), dtype=np.int64)
    for a in range(NQ):
        cols = np.arange(a, NQ)
        bins = np.floor(np.sqrt(a * a + cols.astype(np.float64) ** 2)).astype(np.int64)
        rem[a, cols] = bins - cols
    table = [[] for _ in CHUNKS]
    for t in range(NROUNDS):
        bit = 1 << t
        move = (rem >= 0) & ((rem & bit) != 0)
        for ci, (c0, c1) in enumerate(CHUNKS):
            mv = move[c0:c1]
            cols_any = np.nonzero(mv.any(axis=0))[0]
            if len(cols_any) == 0:
                table[ci].append((0, 0, None))
            else:
                # widen span to even bounds: 4B-aligned bf16 slices let the
                # DVE pick its 2x packed mode
                lo, hi = int(cols_any[0]) & ~1, int(cols_any[-1]) + 1
                hi += hi & 1
                m = np.zeros((128, hi - lo), dtype=np.float32)
                m[: c1 - c0] = mv[:, lo:hi]
                table[ci].append((lo, hi, m))
        new_rem = -np.ones_like(rem)
        stay = (rem >= 0) & ~move
        new_rem[stay] = rem[stay]
        sr, sc = np.nonzero(move)
        dc = sc + bit
        landing = rem[sr, sc] - bit
        cur = new_rem[sr, dc]
        assert ((cur == -1) | (cur == landing)).all()
        new_rem[sr, dc] = landing
        rem = new_rem
    assert (rem[rem >= 0] == 0).all()
    return table


def _host_constants():
    if "consts" in _CACHE:
        return _CACHE["consts"]
    import ml_dtypes

    u = np.arange(H, dtype=np.float64)
    v = np.arange(520, dtype=np.float64)
    ang1 = 2.0 * np.pi * np.outer(u, v) / H
    Cm = np.cos(ang1).astype(np.float32)        # [1024, 520]
    Sm = np.sin(ang1).astype(np.float32)

    mp = np.arange(512, dtype=np.float64)
    w = np.arange(512, dtype=np.float64)
    ae = 2.0 * np.pi * np.outer(mp, w) / 512.0
    ao = 2.0 * np.pi * np.outer(2 * mp + 1, w) / 1024.0
    Ce = np.cos(ae).astype(np.float32)
    Se = np.sin(ae).astype(np.float32)
    Co = np.cos(ao).astype(np.float32)
    So = np.sin(ao).astype(np.float32)

    # radial bin counts exactly as reference._radial_bins (unshifted coords)
    y = np.minimum(np.arange(H), H - np.arange(H))
    yy, xx = np.meshgrid(y, y, indexing="ij")
    dist = np.sqrt((xx.astype(np.float64)) ** 2 + yy.astype(np.float64) ** 2)
    bins_full = np.clip(dist.astype(np.int32), 0, NB - 1)
    counts = np.bincount(bins_full.reshape(-1), minlength=NB).astype(np.float64)
    invc = np.zeros((1, WB), dtype=np.float32)
    invc[0, :NB] = (1.0 / counts).astype(np.float32)

    # row weights w_v for v = 0..512; sw cols: 2*mu = sqrt(w)/H (ACT square
    # scale), 2*mu+1 = 1/w (p2 accumulation), 10+mu = w/H^2 (DVE stt square)
    wv = np.full(NQ, 2.0)
    wv[0] = 1.0
    wv[512] = 1.0
    swc = np.zeros((128, 16), dtype=np.float32)
    for mu in range(5):
        c0, c1 = CHUNKS[mu]
        n = c1 - c0
        swc[:n, 2 * mu] = (np.sqrt(wv[c0:c1]) / H).astype(np.float32)
        swc[:n, 2 * mu + 1] = (1.0 / wv[c0:c1]).astype(np.float32)
        swc[:n, 10 + mu] = (wv[c0:c1] / (H * H)).astype(np.float32)

    table = _gen_barrel_masks()
    chunk_w = [max(1, sum(hi - lo for (lo, hi, m) in table[ci])) for ci in range(5)]
    maxw = max(chunk_w)
    bmask = np.zeros((640, maxw), dtype=np.float32)
    for ci in range(5):
        off = 0
        for (lo, hi, m) in table[ci]:
            if m is None:
                continue
            bmask[128 * ci : 128 * ci + 128, off : off + hi - lo] = m
            off += hi - lo

    # diagonal-block mask: 0 below diag, 0.5 on diag, 1 above (block-local)
    a = np.arange(128)
    mfd = (a[None, :] > a[:, None]).astype(np.float32)
    mfd[a, a] = 0.5

    _CACHE["consts"] = dict(
        Cm=Cm, Sm=Sm, Ce=Ce, Se=Se, Sen=(-Se), Co=Co, So=So, Son=(-So),
        invc=invc, swc=swc,
        bmask_bf16=bmask.astype(ml_dtypes.bfloat16),
        mfd_bf16=mfd.astype(ml_dtypes.bfloat16),
        table=table, maxw=maxw, counts=counts,
    )
    return _CACHE["consts"]


# ---------------------------------------------------------------- device build
def _build_nc():
    hc = _host_constants()
    table, maxw = hc["table"], hc["maxw"]

    nc = bacc.Bacc("TRN2", target_bir_lowering=False, debug=False)
    x_p = nc.declare_dram_parameter("x", [IMGS_PER_CORE, H, H], F32R, isOutput=False)
    cm_p = nc.declare_dram_parameter("cm", [H, 520], F32R, isOutput=False)
    sm_p = nc.declare_dram_parameter("sm", [H, 520], F32R, isOutput=False)
    ce_p = nc.declare_dram_parameter("ce", [512, 512], F32R, isOutput=False)
    se_p = nc.declare_dram_parameter("se", [512, 512], F32R, isOutput=False)
    sen_p = nc.declare_dram_parameter("sen", [512, 512], F32R, isOutput=False)
    co_p = nc.declare_dram_parameter("co", [512, 512], F32R, isOutput=False)
    so_p = nc.declare_dram_parameter("so", [512, 512], F32R, isOutput=False)
    son_p = nc.declare_dram_parameter("son", [512, 512], F32R, isOutput=False)
    bm_p = nc.declare_dram_parameter("bm", [640, maxw], BF16, isOutput=False)
    md_p = nc.declare_dram_parameter("md", [128, 128], BF16, isOutput=False)
    sw_p = nc.declare_dram_parameter("sw", [128, 16], F32, isOutput=False)
    ic_p = nc.declare_dram_parameter("ic", [1, WB], F32, isOutput=False)
    out_p = nc.declare_dram_parameter("out", [1, IMGS_PER_CORE], F32, isOutput=True)

    AT = mybir.AluOpType

    with tile.TileContext(nc) as tc:
        with (
            tc.tile_pool(name="const", bufs=1) as cpool,
            tc.tile_pool(name="xin", bufs=1) as xpool,
            tc.tile_pool(name="arr", bufs=1) as apool,
            tc.tile_pool(name="quad", bufs=1) as qpool,
            tc.tile_pool(name="work", bufs=2) as wpool,
            tc.tile_pool(name="ps", bufs=2, space="PSUM") as ps,
        ):
            # ---------------- constants
            Cm_t = [cpool.tile([128, 520], F32R, tag=f"cm{k}", name=f"cm{k}") for k in range(8)]
            Sm_t = [cpool.tile([128, 520], F32R, tag=f"sm{k}", name=f"sm{k}") for k in range(8)]
            CeT = [cpool.tile([128, 512], F32R, tag=f"ce{k}", name=f"ce{k}") for k in range(4)]
            SeT = [cpool.tile([128, 512], F32R, tag=f"sE{k}", name=f"sE{k}") for k in range(4)]
            SenT = [cpool.tile([128, 512], F32R, tag=f"sn{k}", name=f"sn{k}") for k in range(4)]
            CoT = [cpool.tile([128, 512], F32R, tag=f"co{k}", name=f"co{k}") for k in range(4)]
            SoT = [cpool.tile([128, 512], F32R, tag=f"sO{k}", name=f"sO{k}") for k in range(4)]
            SonT = [cpool.tile([128, 512], F32R, tag=f"sm{k}b", name=f"sm{k}b") for k in range(4)]
            bm_t = [
                cpool.tile([128, max(1, sum(hi - lo for (lo, hi, m) in table[ci]))],
                           BF16, tag=f"bm{ci}", name=f"bm{ci}")
                for ci in range(5)
            ]
            mfd_t = cpool.tile([128, 128], BF16, tag="mfd")
            sw_t = cpool.tile([128, 16], F32, tag="sw")
            ic_t = cpool.tile([1, WB], F32, tag="ic")
            ident = cpool.tile([128, 128], F32, tag="ident")
            make_identity(nc, ident[:])
            identr = cpool.tile([128, 128], F32R, tag="identr")
            nc.vector.tensor_copy(ident[:], ident[:])
            ones32 = cpool.tile([128, 1], F32, tag="ones32")
            nc.gpsimd.memset(ones32[:], 1.0)
            ones = cpool.tile([128, 1], F32R, tag="ones")
            nc.vector.tensor_copy(ones[:], ones32[:])
            onesb = cpool.tile([128, 1], BF16, tag="onesb")
            nc.vector.tensor_copy(onesb[:], ones32[:])
            zt = cpool.tile([128, 8], F32, tag="zt")
            nc.gpsimd.memset(zt[:], 0.0)
            lossv = cpool.tile([1, IMGS_PER_CORE], F32, tag="lossv")

            # ---------------- per-image persistent arrays
            Xt = [xpool.tile([128, H], F32R, tag=f"x{k}", name=f"x{k}") for k in range(8)]
            # A blocks: j=0..3 even m' chunks, j=4..7 odd m' chunks
            Ar = [apool.tile([128, NQ], F32R, tag=f"ar{j}", name=f"ar{j}") for j in range(8)]
            Ai = [apool.tile([128, NQ], F32R, tag=f"ai{j}", name=f"ai{j}") for j in range(8)]
            # f32r folded-G strips (shared across images) + bf16 barrel strips
            # double-buffered across images (for the pipelined red)
            Gq = [qpool.tile([128, NQ], F32, tag=f"gq{ci}", name=f"gq{ci}")
                  for ci in range(5)]
            Xb = [[qpool.tile([128, WB], BF16, tag=f"xb{p}_{ci}", name=f"xb{p}_{ci}")
                   for ci in range(5)] for p in range(IMGS_PER_CORE)]
            P2 = [qpool.tile([128, 8], F32R, tag=f"p2acc{p}", name=f"p2acc{p}")
                  for p in range(IMGS_PER_CORE)]

            def s1(img):
                """first DFT: fills Ar/Ai blocks; Ai = -Im(A)"""
                for m in range(8):
                    pr_lo = ps.tile([128, 512], F32, tag="pa")
                    pr_hi = ps.tile([128, 8], F32, tag="pd")
                    pt_lo = ps.tile([128, 512], F32, tag="pb")
                    for k in range(8):
                        lhs = Xt[k][:, 128 * m : 128 * m + 128]
                        st, sp = (k == 0), (k == 7)
                        nc.tensor.matmul(pr_lo[:], lhs, Cm_t[k][:, 0:512], start=st, stop=sp)
                        nc.tensor.matmul(pr_hi[:], lhs, Cm_t[k][:, 512:520], start=st, stop=sp)
                        nc.tensor.matmul(pt_lo[:], lhs, Sm_t[k][:, 0:512], start=st, stop=sp)
                    nc.scalar.activation(Ar[m][:, 0:512], pr_lo[:], AF.Copy)
                    nc.scalar.activation(Ar[m][:, 512:513], pr_hi[:, 0:1], AF.Copy)
                    nc.scalar.activation(Ai[m][:, 0:512], pt_lo[:], AF.Copy)
                    nc.vector.tensor_copy(Ai[m][:, 512:513], zt[:, 0:1])

            def s2pre(img):
                """zero barrel strips, the strip-4 G row, and p2acc"""
                xb = Xb[img]
                for ci in range(5):
                    nc.gpsimd.memset(xb[ci][:], 0.0)
                nc.gpsimd.memset(Gq[4][:], 0.0)
                nc.vector.tensor_copy(P2[img][:], zt[:])

            def s2row(img, mu):
                """second DFT (even/odd split) + power + fold, one v-block"""
                p2acc = P2[img]
                if True:
                    M = 128 if mu < 4 else 1
                    u0 = 128 * mu
                    pfer = ps.tile([128, 512], F32, tag="pa")
                    pfor = ps.tile([128, 512], F32, tag="pb")
                    for k in range(4):
                        st, sp = (k == 0), (k == 3)
                        er = Ar[k][:, u0 : u0 + M]
                        ei = Ai[k][:, u0 : u0 + M]
                        orr = Ar[4 + k][:, u0 : u0 + M]
                        oi = Ai[4 + k][:, u0 : u0 + M]
                        if mu < 4:
                            nc.tensor.matmul(pfer[0:M], er, CeT[k][:], start=st, stop=False)
                            nc.tensor.matmul(pfer[0:M], ei, SenT[k][:], start=False, stop=sp,
                                             skip_group_check=True)
                            nc.tensor.matmul(pfor[0:M], orr, CoT[k][:], start=st, stop=False)
                            nc.tensor.matmul(pfor[0:M], oi, SonT[k][:], start=False, stop=sp,
                                             skip_group_check=True)
                        else:
                            nc.tensor.matmul(pfer[0:M], er, CeT[k][:], start=st, stop=sp)
                            nc.tensor.matmul(pfor[0:M], orr, CoT[k][:], start=st, stop=sp)
                    sc_ap = sw_t[0:M, 2 * mu : 2 * mu + 1]
                    feR = wpool.tile([128, 512], F32, tag="feR", bufs=1)
                    zrp = wpool.tile([128, 512], F32, tag="zrp", bufs=1)
                    zrm = wpool.tile([128, 512], F32, tag="zrm", bufs=1)
                    nc.scalar.activation(feR[0:M], pfer[0:M], AF.Copy, scale=sc_ap)
                    nc.vector.scalar_tensor_tensor(
                        zrp[0:M], pfor[0:M], sc_ap, feR[0:M], op0=AT.mult, op1=AT.add)
                    nc.vector.scalar_tensor_tensor(
                        zrm[0:M], pfor[0:M], sc_ap, feR[0:M], op0=AT.mult, op1=AT.subtract)
                    pnei = ps.tile([128, 512], F32, tag="pa")
                    pnoi = ps.tile([128, 512], F32, tag="pb")
                    for k in range(4):
                        st, sp = (k == 0), (k == 3)
                        er = Ar[k][:, u0 : u0 + M]
                        ei = Ai[k][:, u0 : u0 + M]
                        orr = Ar[4 + k][:, u0 : u0 + M]
                        oi = Ai[4 + k][:, u0 : u0 + M]
                        if mu < 4:
                            nc.tensor.matmul(pnei[0:M], ei, CeT[k][:], start=st, stop=False)
                            nc.tensor.matmul(pnei[0:M], er, SeT[k][:], start=False, stop=sp,
                                             skip_group_check=True)
                            nc.tensor.matmul(pnoi[0:M], oi, CoT[k][:], start=st, stop=False)
                            nc.tensor.matmul(pnoi[0:M], orr, SoT[k][:], start=False, stop=sp,
                                             skip_group_check=True)
                        else:
                            nc.tensor.matmul(pnei[0:M], er, SeT[k][:], start=st, stop=sp)
                            nc.tensor.matmul(pnoi[0:M], orr, SoT[k][:], start=st, stop=sp)
                    feI = wpool.tile([128, 512], F32, tag="feI", bufs=1)
                    zip_ = wpool.tile([128, 512], F32, tag="zip", bufs=1)
                    zim = wpool.tile([128, 512], F32, tag="zim", bufs=1)
                    nc.scalar.activation(feI[0:M], pnei[0:M], AF.Copy, scale=sc_ap)
                    nc.vector.scalar_tensor_tensor(
                        zip_[0:M], pnoi[0:M], sc_ap, feI[0:M], op0=AT.mult, op1=AT.add)
                    nc.vector.scalar_tensor_tensor(
                        zim[0:M], pnoi[0:M], sc_ap, feI[0:M], op0=AT.mult, op1=AT.subtract)

                    # U+/- = |sc*Z|^2 (scale already folded into Z);
                    # computed in place: up aliases zrp, um aliases zrm
                    up, um = zrp, zrm
                    nc.scalar.activation(up[0:M], zrp[0:M], AF.Square)
                    nc.scalar.activation(zip_[0:M], zip_[0:M], AF.Square)
                    nc.vector.tensor_tensor(out=up[0:M], in0=up[0:M], in1=zip_[0:M], op=AT.add)
                    nc.scalar.activation(zrm[0:M], zrm[0:M], AF.Square)
                    nc.scalar.activation(zim[0:M], zim[0:M], AF.Square)
                    nc.vector.tensor_tensor(out=um[0:M], in0=um[0:M], in1=zim[0:M], op=AT.add)
                    if mu == 0:
                        # zero DC: kills the catastrophic p2/q2 cancellation
                        nc.vector.tensor_copy(up[0:1, 0:1], zt[0:1, 0:1])
                    # fold to G strip: G[w']=U+(w')+U-(512-w')
                    nc.vector.tensor_tensor(
                        out=Gq[mu][0:M, 1:512], in0=up[0:M, 1:512],
                        in1=um[0:M, 511:0:-1], op=AT.add)
                    nc.vector.tensor_copy(Gq[mu][0:M, 0:1], up[0:M, 0:1])
                    nc.vector.tensor_copy(Gq[mu][0:M, 512:513], um[0:M, 0:1])
                    # p2 accumulation
                    rsp = wpool.tile([128, 1], F32, tag="rsp")
                    rsm = wpool.tile([128, 1], F32, tag="rsm")
                    nc.scalar.activation(up[0:M], up[0:M], AF.Square, accum_out=rsp[0:M])
                    nc.scalar.activation(um[0:M], um[0:M], AF.Square, accum_out=rsm[0:M])
                    nc.vector.scalar_tensor_tensor(
                        p2acc[0:M, 0:1], rsp[0:M], sw_t[0:M, 2 * mu + 1 : 2 * mu + 2],
                        p2acc[0:M, 0:1], op0=AT.mult, op1=AT.add)
                    nc.vector.scalar_tensor_tensor(
                        p2acc[0:M, 0:1], rsm[0:M], sw_t[0:M, 2 * mu + 1 : 2 * mu + 2],
                        p2acc[0:M, 0:1], op0=AT.mult, op1=AT.add)

            def dfrow(img, ci):
                """diagonal fold G + G^T (upper triangle) via PSUM accumulate,
                writing the bf16 barrel strips directly; one destination strip"""
                xb = Xb[img]
                if True:
                    for cj in range(ci, 4):
                        tp = ps.tile([128, 128], F32, tag="pd")
                        if ci == cj:
                            nc.tensor.matmul(tp[:], ident[:],
                                             Gq[ci][:, 128 * cj : 128 * cj + 128],
                                             start=True, stop=False)
                            nc.tensor.matmul(tp[:], Gq[cj][:, 128 * ci : 128 * ci + 128],
                                             ident[:], is_transpose=True,
                                             start=False, stop=True,
                                             skip_group_check=True)
                            nc.vector.tensor_tensor(
                                out=xb[ci][:, 128 * cj : 128 * cj + 128],
                                in0=tp[:], in1=mfd_t[:], op=AT.mult)
                        else:
                            nc.tensor.matmul(tp[:], Gq[cj][:, 128 * ci : 128 * ci + 128],
                                             ident[:], is_transpose=True,
                                             start=True, stop=True)
                            nc.vector.tensor_tensor(
                                out=xb[ci][:, 128 * cj : 128 * cj + 128],
                                in0=tp[:], in1=Gq[ci][:, 128 * cj : 128 * cj + 128],
                                op=AT.add)
                    # column 512 += transpose of strip-4 block
                    tp4 = ps.tile([128, 128], F32, tag="pd")
                    nc.tensor.matmul(tp4[:], Gq[4][:, 128 * ci : 128 * ci + 128],
                                     ident[:], is_transpose=True,
                                     start=True, stop=True)
                    nc.vector.tensor_tensor(
                        out=xb[ci][:, 512:513], in0=tp4[:, 0:1],
                        in1=Gq[ci][:, 512:513], op=AT.add)

            def brlstrip(img, ci):
                """barrel shear: align each row's columns to radial bins"""
                xb = Xb[img]
                if True:
                    off = 0
                    for t in range(NROUNDS):
                        lo, hi, m = table[ci][t]
                        wdt = hi - lo
                        if wdt <= 0:
                            continue
                        bit = 1 << t
                        gps = ci >= 3
                        eng = nc.gpsimd if gps else nc.vector
                        tmp = wpool.tile([128, 192 if gps else 512], BF16,
                                         tag=("btmpg" if gps else "btmp"), bufs=1)
                        eng.tensor_tensor(
                            out=tmp[:, 0:wdt], in0=xb[ci][:, lo:hi],
                            in1=bm_t[ci][:, off : off + wdt], op=AT.mult)
                        eng.tensor_tensor(
                            out=xb[ci][:, lo:hi], in0=xb[ci][:, lo:hi],
                            in1=tmp[:, 0:wdt], op=AT.subtract)
                        eng.tensor_tensor(
                            out=xb[ci][:, lo + bit : hi + bit],
                            in0=xb[ci][:, lo + bit : hi + bit],
                            in1=tmp[:, 0:wdt], op=AT.add)
                        off += wdt

            def red(img):
                """per-bin sums -> q2; loss = p2 - q2"""
                xb = Xb[img]
                ps_lo = ps.tile([1, 512], F32, tag="pa")
                ps_hi = ps.tile([1, 216], F32, tag="pb")
                for ci in range(5):
                    st, sp = (ci == 0), (ci == 4)
                    nc.tensor.matmul(ps_lo[:], onesb[:], xb[ci][:, 0:512], start=st, stop=sp)
                    nc.tensor.matmul(ps_hi[:], onesb[:], xb[ci][:, 512:WB], start=st, stop=sp)
                ssq = wpool.tile([1, WB], F32, tag="ssq", bufs=1)
                nc.scalar.activation(ssq[0:1, 0:512], ps_lo[:], AF.Square)
                nc.scalar.activation(ssq[0:1, 512:WB], ps_hi[:], AF.Square)
                nc.vector.tensor_tensor(out=ssq[:], in0=ssq[:], in1=ic_t[:], op=AT.mult)
                q2 = wpool.tile([1, 1], F32, tag="q2")
                nc.vector.tensor_reduce(q2[:], ssq[:], axis=mybir.AxisListType.X, op=AT.add)
                psp = ps.tile([1, 8], F32, tag="pd")
                nc.tensor.matmul(psp[:], ones[:], P2[img][:], start=True, stop=True)
                nc.vector.tensor_tensor(
                    out=lossv[0:1, img : img + 1], in0=psp[0:1, 0:1], in1=q2[:],
                    op=AT.subtract)

            def s2df(img):
                """s2 + fold + barrel, v-blocks descending so each strip's
                barrel overlaps the remaining blocks' matmuls"""
                s2pre(img)
                s2row(img, 4)
                # strip 4: the lone (512,512) diagonal cell keeps weight 1
                nc.scalar.activation(Xb[img][4][0:1, 512:513],
                                     Gq[4][0:1, 512:513], AF.Copy)
                brlstrip(img, 4)
                for mu in (3, 2, 1, 0):
                    s2row(img, mu)
                    dfrow(img, mu)
                    brlstrip(img, mu)

            # ---------------- program (software-pipelined over 2 images)
            # PE warmup: keep HAM un-throttled while the input/const DMAs
            # stream in (PE would otherwise idle ~20us and start cold)
            for i in range(96):
                pw = ps.tile([128, 128], F32, tag="pd")
                nc.tensor.matmul(pw[:], ident[:], ident[:], start=True, stop=True)

            for k in range(8):
                nc.sync.dma_start(Xt[k][:], x_p[0, 128 * k : 128 * k + 128, :])
            for k in range(8):
                nc.sync.dma_start(Cm_t[k][:], cm_p[128 * k : 128 * k + 128, :])
                nc.sync.dma_start(Sm_t[k][:], sm_p[128 * k : 128 * k + 128, :])
            nc.sync.dma_start(sw_t[:], sw_p[:])
            nc.sync.dma_start(ic_t[:], ic_p[:])
            for k in range(4):
                sl = slice(128 * k, 128 * k + 128)
                nc.sync.dma_start(CeT[k][:], ce_p[sl, :])
                nc.sync.dma_start(SeT[k][:], se_p[sl, :])
                nc.sync.dma_start(SenT[k][:], sen_p[sl, :])
                nc.sync.dma_start(CoT[k][:], co_p[sl, :])
                nc.sync.dma_start(SoT[k][:], so_p[sl, :])
                nc.sync.dma_start(SonT[k][:], son_p[sl, :])

            sc = nc.named_scope("s1_0"); sc.__enter__()
            s1(0)
            sc.__exit__(None, None, None)
            # prefetch image 1 and the barrel masks behind the s1 constants
            for k in range(8):
                nc.sync.dma_start(Xt[k][:], x_p[1, 128 * k : 128 * k + 128, :])
            for ci in range(5):
                wci = sum(hi - lo for (lo, hi, m) in table[ci])
                if wci > 0:
                    nc.sync.dma_start(bm_t[ci][:, 0:wci],
                                      bm_p[128 * ci : 128 * ci + 128, 0:wci])
            nc.sync.dma_start(mfd_t[:], md_p[:])

            sc = nc.named_scope("s2_0"); sc.__enter__()
            s2df(0)
            sc.__exit__(None, None, None)
            sc = nc.named_scope("s1_1"); sc.__enter__()
            s1(1)
            sc.__exit__(None, None, None)
            sc = nc.named_scope("red_0"); sc.__enter__()
            red(0)
            sc.__exit__(None, None, None)
            sc = nc.named_scope("s2_1"); sc.__enter__()
            s2df(1)
            sc.__exit__(None, None, None)
            sc = nc.named_scope("red_1"); sc.__enter__()
            red(1)
            sc.__exit__(None, None, None)

            nc.sync.dma_start(out_p[:], lossv[:])

    nc.compile()
    return nc


def _get_nc():
    if "nc" not in _CACHE:
        _CACHE["nc"] = _build_nc()
    return _CACHE["nc"]


# ---------------------------------------------------------------- entry point
def kernel(prob_cg: np.ndarray) -> np.ndarray:
    hc = _host_constants()
    nc = _get_nc()
    x = prob_cg[:, 0, :, :].astype(np.float32)
    # pre-permute columns to [even m | odd m] so s1 writes even/odd A blocks
    xp = np.ascontiguousarray(
        np.concatenate([x[:, :, 0::2], x[:, :, 1::2]], axis=2))
    in_maps = []
    for i in range(N_CORES):
        in_maps.append(
            dict(
                x=xp[2 * i : 2 * i + 2],
                cm=hc["Cm"], sm=hc["Sm"],
                ce=hc["Ce"], se=hc["Se"], sen=hc["Sen"],
                co=hc["Co"], so=hc["So"], son=hc["Son"],
                bm=hc["bmask_bf16"], md=hc["mfd_bf16"],
                sw=hc["swc"], ic=hc["invc"],
            )
        )
    trace = os.environ.get("AT_TRACE", "0") == "1"
    kw = {}
    if trace and os.environ.get("AT_TMPDIR"):
        kw["tmpdir"] = os.environ["AT_TMPDIR"]
    res = run_bass_kernel_spmd(nc, in_maps, core_ids=list(range(N_CORES)), trace=trace, **kw)
    if trace and res.exec_time_ns is not None:
        print(f"HW exec time: {res.exec_time_ns} ns")
        if res.per_core_scope_times:
            for kname, v in sorted(res.per_core_scope_times.items()):
                print(f"  scope {kname}: {v}")
    losses = np.concatenate([r["out"].reshape(-1) for r in res.results])
    loss = losses.mean() + (H * H) * (EPS * EPS)
    return np.float32(WA * loss)


# revision 38
# speedup vs baseline: 1.0396x; 1.0080x over previous
"""AnisotropySuppressionLoss on 8 TRN2 NeuronCores (Bass/Tile), v2.

Per image (1024x1024 fp32):
  s1: A[m, v] = DFT_u x[u, m], v = 0..512 (real-input half spectrum),
      via f32r matmuls. Host pre-permutes image columns to [even m | odd m]
      so A lands in even/odd m' blocks.
  s2: radix-2 over m with the twiddle absorbed into the odd-m DFT matrix:
      Fe = DFT_{even m}(A), Fo' = sum_{odd m} A e^{-2pi i m w/1024};
      Z(w) = Fe + Fo', Z(w+512) = Fe - Fo' (w = 0..511). Halves matmul work
      vs a direct 1024-wide second DFT; butterflies are 4 DVE adds/block.
  power: U+/- = w_v/H^2 * |Z|^2 via ACT squares (scale) + DVE stt squares.
  fold:  G[v,w'] = U+(w') + U-(512-w') into bf16 strips; diagonal fold
      G + G^T via PE transpose accumulated in PSUM (ACT copy back).
  radial: barrel-shift shear (exact), bf16; strips 0-2 on DVE,
      3-4 on GpSimd; PE warmup stream covers the initial DMA window.
  loss_img = sum w*P^2/w - sum_b S_b^2/c_b  (+ H*W*eps^2 on host).
Data-parallel: batch 16 -> 2 images/core on 8 cores; host averages.
"""

import os
import sys

sys.path.insert(0, "/opt/trn_rl_repo")

import numpy as np

import concourse.bass as bass
import concourse.tile as tile
from concourse import bacc, mybir
from concourse.bass_utils import run_bass_kernel_spmd
from concourse.masks import make_identity

F32 = mybir.dt.float32
F32R = mybir.dt.float32r
BF16 = mybir.dt.bfloat16

H = 1024
NQ = 513          # quadrant size (|du|, |dv| in 0..512)
NB = 725          # radial bins 0..724
WB = 728          # barrel buffer width
NROUNDS = 8
N_CORES = 8
IMGS_PER_CORE = 2
WA = 0.002
EPS = 1e-12
CHUNKS = [(0, 128), (128, 256), (256, 384), (384, 512), (512, 513)]
AF = mybir.ActivationFunctionType

_CACHE = {}


# ---------------------------------------------------------------- host consts
def _gen_barrel_masks():
    """Per (chunk, round): (lo, hi, move_mask[128, hi-lo]) in quadrant coords.
    Cells (a, b>=a) carry delta = bin - b; each round moves cells with bit t
    set right by 2^t. Merges are exact (same remaining delta)."""
    rem = -np.ones((NQ, WB), dtype=np.int64)
    for a in range(NQ):
        cols = np.arange(a, NQ)
        bins = np.floor(np.sqrt(a * a + cols.astype(np.float64) ** 2)).astype(np.int64)
        rem[a, cols] = bins - cols
    table = [[] for _ in CHUNKS]
    for t in range(NROUNDS):
        bit = 1 << t
        move = (rem >= 0) & ((rem & bit) != 0)
        for ci, (c0, c1) in enumerate(CHUNKS):
            mv = move[c0:c1]
            cols_any = np.nonzero(mv.any(axis=0))[0]
            if len(cols_any) == 0:
                table[ci].append((0, 0, None))
            else:
                # widen span to even bounds: 4B-aligned bf16 slices let the
                # DVE pick its 2x packed mode
                lo, hi = int(cols_any[0]) & ~1, int(cols_any[-1]) + 1
                hi += hi & 1
                m = np.zeros((128, hi - lo), dtype=np.float32)
                m[: c1 - c0] = mv[:, lo:hi]
                table[ci].append((lo, hi, m))
        new_rem = -np.ones_like(rem)
        stay = (rem >= 0) & ~move
        new_rem[stay] = rem[stay]
        sr, sc = np.nonzero(move)
        dc = sc + bit
        landing = rem[sr, sc] - bit
        cur = new_rem[sr, dc]
        assert ((cur == -1) | (cur == landing)).all()
        new_rem[sr, dc] = landing
        rem = new_rem
    assert (rem[rem >= 0] == 0).all()
    return table


def _host_constants():
    if "consts" in _CACHE:
        return _CACHE["consts"]
    import ml_dtypes

    u = np.arange(H, dtype=np.float64)
    v = np.arange(520, dtype=np.float64)
    ang1 = 2.0 * np.pi * np.outer(u, v) / H
    Cm = np.cos(ang1).astype(np.float32)        # [1024, 520]
    Sm = np.sin(ang1).astype(np.float32)

    mp = np.arange(512, dtype=np.float64)
    w = np.arange(512, dtype=np.float64)
    ae = 2.0 * np.pi * np.outer(mp, w) / 512.0
    ao = 2.0 * np.pi * np.outer(2 * mp + 1, w) / 1024.0
    Ce = np.cos(ae).astype(np.float32)
    Se = np.sin(ae).astype(np.float32)
    Co = np.cos(ao).astype(np.float32)
    So = np.sin(ao).astype(np.float32)

    # radial bin counts exactly as reference._radial_bins (unshifted coords)
    y = np.minimum(np.arange(H), H - np.arange(H))
    yy, xx = np.meshgrid(y, y, indexing="ij")
    dist = np.sqrt((xx.astype(np.float64)) ** 2 + yy.astype(np.float64) ** 2)
    bins_full = np.clip(dist.astype(np.int32), 0, NB - 1)
    counts = np.bincount(bins_full.reshape(-1), minlength=NB).astype(np.float64)
    invc = np.zeros((1, WB), dtype=np.float32)
    invc[0, :NB] = (1.0 / counts).astype(np.float32)

    # row weights w_v for v = 0..512; sw cols: 2*mu = sqrt(w)/H (ACT square
    # scale), 2*mu+1 = 1/w (p2 accumulation), 10+mu = w/H^2 (DVE stt square)
    wv = np.full(NQ, 2.0)
    wv[0] = 1.0
    wv[512] = 1.0
    swc = np.zeros((128, 16), dtype=np.float32)
    for mu in range(5):
        c0, c1 = CHUNKS[mu]
        n = c1 - c0
        swc[:n, 2 * mu] = (np.sqrt(wv[c0:c1]) / H).astype(np.float32)
        swc[:n, 2 * mu + 1] = (1.0 / wv[c0:c1]).astype(np.float32)
        swc[:n, 10 + mu] = (-np.sqrt(wv[c0:c1]) / H).astype(np.float32)

    table = _gen_barrel_masks()
    chunk_w = [max(1, sum(hi - lo for (lo, hi, m) in table[ci])) for ci in range(5)]
    maxw = max(chunk_w)
    bmask = np.zeros((640, maxw), dtype=np.float32)
    for ci in range(5):
        off = 0
        for (lo, hi, m) in table[ci]:
            if m is None:
                continue
            bmask[128 * ci : 128 * ci + 128, off : off + hi - lo] = m
            off += hi - lo

    # diagonal-block mask: 0 below diag, 0.5 on diag, 1 above (block-local)
    a = np.arange(128)
    mfd = (a[None, :] > a[:, None]).astype(np.float32)
    mfd[a, a] = 0.5

    _CACHE["consts"] = dict(
        Cm=Cm, Sm=Sm, Ce=Ce, Se=Se, Sen=(-Se), Co=Co, So=So, Son=(-So),
        invc=invc, swc=swc,
        bmask_bf16=bmask.astype(ml_dtypes.bfloat16),
        mfd_bf16=mfd.astype(ml_dtypes.bfloat16),
        table=table, maxw=maxw, counts=counts,
    )
    return _CACHE["consts"]


# ---------------------------------------------------------------- device build
def _build_nc():
    hc = _host_constants()
    table, maxw = hc["table"], hc["maxw"]

    nc = bacc.Bacc("TRN2", target_bir_lowering=False, debug=False)
    x_p = nc.declare_dram_parameter("x", [IMGS_PER_CORE, H, H], F32R, isOutput=False)
    cm_p = nc.declare_dram_parameter("cm", [H, 520], F32R, isOutput=False)
    sm_p = nc.declare_dram_parameter("sm", [H, 520], F32R, isOutput=False)
    ce_p = nc.declare_dram_parameter("ce", [512, 512], F32R, isOutput=False)
    se_p = nc.declare_dram_parameter("se", [512, 512], F32R, isOutput=False)
    sen_p = nc.declare_dram_parameter("sen", [512, 512], F32R, isOutput=False)
    co_p = nc.declare_dram_parameter("co", [512, 512], F32R, isOutput=False)
    so_p = nc.declare_dram_parameter("so", [512, 512], F32R, isOutput=False)
    son_p = nc.declare_dram_parameter("son", [512, 512], F32R, isOutput=False)
    bm_p = nc.declare_dram_parameter("bm", [640, maxw], BF16, isOutput=False)
    md_p = nc.declare_dram_parameter("md", [128, 128], BF16, isOutput=False)
    sw_p = nc.declare_dram_parameter("sw", [128, 16], F32, isOutput=False)
    ic_p = nc.declare_dram_parameter("ic", [1, WB], F32, isOutput=False)
    out_p = nc.declare_dram_parameter("out", [1, IMGS_PER_CORE], F32, isOutput=True)

    AT = mybir.AluOpType

    with tile.TileContext(nc) as tc:
        with (
            tc.tile_pool(name="const", bufs=1) as cpool,
            tc.tile_pool(name="xin", bufs=1) as xpool,
            tc.tile_pool(name="arr", bufs=1) as apool,
            tc.tile_pool(name="quad", bufs=1) as qpool,
            tc.tile_pool(name="work", bufs=2) as wpool,
            tc.tile_pool(name="ps", bufs=2, space="PSUM") as ps,
        ):
            # ---------------- constants
            Cm_t = [cpool.tile([128, 520], F32R, tag=f"cm{k}", name=f"cm{k}") for k in range(8)]
            Sm_t = [cpool.tile([128, 520], F32R, tag=f"sm{k}", name=f"sm{k}") for k in range(8)]
            CeT = [cpool.tile([128, 512], F32R, tag=f"ce{k}", name=f"ce{k}") for k in range(4)]
            SeT = [cpool.tile([128, 512], F32R, tag=f"sE{k}", name=f"sE{k}") for k in range(4)]
            SenT = [cpool.tile([128, 512], F32R, tag=f"sn{k}", name=f"sn{k}") for k in range(4)]
            CoT = [cpool.tile([128, 512], F32R, tag=f"co{k}", name=f"co{k}") for k in range(4)]
            SoT = [cpool.tile([128, 512], F32R, tag=f"sO{k}", name=f"sO{k}") for k in range(4)]
            SonT = [cpool.tile([128, 512], F32R, tag=f"sm{k}b", name=f"sm{k}b") for k in range(4)]
            bm_t = [
                cpool.tile([128, max(1, sum(hi - lo for (lo, hi, m) in table[ci]))],
                           BF16, tag=f"bm{ci}", name=f"bm{ci}")
                for ci in range(5)
            ]
            mfd_t = cpool.tile([128, 128], BF16, tag="mfd")
            sw_t = cpool.tile([128, 16], F32, tag="sw")
            ic_t = cpool.tile([1, WB], F32, tag="ic")
            ident = cpool.tile([128, 128], F32, tag="ident")
            make_identity(nc, ident[:])
            identr = cpool.tile([128, 128], F32R, tag="identr")
            nc.vector.tensor_copy(ident[:], ident[:])
            ones32 = cpool.tile([128, 1], F32, tag="ones32")
            nc.gpsimd.memset(ones32[:], 1.0)
            ones = cpool.tile([128, 1], F32R, tag="ones")
            nc.vector.tensor_copy(ones[:], ones32[:])
            onesb = cpool.tile([128, 1], BF16, tag="onesb")
            nc.vector.tensor_copy(onesb[:], ones32[:])
            zt = cpool.tile([128, 8], F32, tag="zt")
            nc.gpsimd.memset(zt[:], 0.0)
            lossv = cpool.tile([1, IMGS_PER_CORE], F32, tag="lossv")

            # ---------------- per-image persistent arrays
            Xt = [xpool.tile([128, H], F32R, tag=f"x{k}", name=f"x{k}") for k in range(8)]
            # A blocks: j=0..3 even m' chunks, j=4..7 odd m' chunks
            Ar = [apool.tile([128, NQ], F32R, tag=f"ar{j}", name=f"ar{j}") for j in range(8)]
            Ai = [apool.tile([128, NQ], F32R, tag=f"ai{j}", name=f"ai{j}") for j in range(8)]
            # f32r folded-G strips (shared across images) + bf16 barrel strips
            # double-buffered across images (for the pipelined red)
            Gq = [qpool.tile([128, NQ], F32, tag=f"gq{ci}", name=f"gq{ci}")
                  for ci in range(5)]
            Xb = [[qpool.tile([128, WB], BF16, tag=f"xb{p}_{ci}", name=f"xb{p}_{ci}")
                   for ci in range(5)] for p in range(IMGS_PER_CORE)]
            P2 = [qpool.tile([128, 8], F32R, tag=f"p2acc{p}", name=f"p2acc{p}")
                  for p in range(IMGS_PER_CORE)]

            def s1(img):
                """first DFT: fills Ar/Ai blocks; Ai = -Im(A)"""
                for m in range(8):
                    pr_lo = ps.tile([128, 512], F32, tag="pa")
                    pr_hi = ps.tile([128, 8], F32, tag="pd")
                    pt_lo = ps.tile([128, 512], F32, tag="pb")
                    for k in range(8):
                        lhs = Xt[k][:, 128 * m : 128 * m + 128]
                        st, sp = (k == 0), (k == 7)
                        nc.tensor.matmul(pr_lo[:], lhs, Cm_t[k][:, 0:512], start=st, stop=sp)
                        nc.tensor.matmul(pr_hi[:], lhs, Cm_t[k][:, 512:520], start=st, stop=sp)
                        nc.tensor.matmul(pt_lo[:], lhs, Sm_t[k][:, 0:512], start=st, stop=sp)
                    nc.scalar.activation(Ar[m][:, 0:512], pr_lo[:], AF.Copy)
                    nc.scalar.activation(Ar[m][:, 512:513], pr_hi[:, 0:1], AF.Copy)
                    nc.scalar.activation(Ai[m][:, 0:512], pt_lo[:], AF.Copy)
                    nc.vector.tensor_copy(Ai[m][:, 512:513], zt[:, 0:1])

            def s2pre(img):
                """zero barrel strips, the strip-4 G row, and p2acc"""
                xb = Xb[img]
                for ci in range(5):
                    nc.gpsimd.memset(xb[ci][:], 0.0)
                nc.gpsimd.memset(Gq[4][:], 0.0)
                nc.vector.tensor_copy(P2[img][:], zt[:])

            def s2row(img, mu):
                """second DFT (even/odd split) + power + fold, one v-block"""
                p2acc = P2[img]
                if True:
                    M = 128 if mu < 4 else 1
                    u0 = 128 * mu
                    pfer = ps.tile([128, 512], F32, tag="pa")
                    pfor = ps.tile([128, 512], F32, tag="pb")
                    for k in range(4):
                        st, sp = (k == 0), (k == 3)
                        er = Ar[k][:, u0 : u0 + M]
                        ei = Ai[k][:, u0 : u0 + M]
                        orr = Ar[4 + k][:, u0 : u0 + M]
                        oi = Ai[4 + k][:, u0 : u0 + M]
                        if mu < 4:
                            nc.tensor.matmul(pfer[0:M], er, CeT[k][:], start=st, stop=False)
                            nc.tensor.matmul(pfer[0:M], ei, SenT[k][:], start=False, stop=sp,
                                             skip_group_check=True)
                            nc.tensor.matmul(pfor[0:M], orr, CoT[k][:], start=st, stop=False)
                            nc.tensor.matmul(pfor[0:M], oi, SonT[k][:], start=False, stop=sp,
                                             skip_group_check=True)
                        else:
                            nc.tensor.matmul(pfer[0:M], er, CeT[k][:], start=st, stop=sp)
                            nc.tensor.matmul(pfor[0:M], orr, CoT[k][:], start=st, stop=sp)
                    sc_ap = sw_t[0:M, 2 * mu : 2 * mu + 1]
                    feR = wpool.tile([128, 512], F32, tag="feR", bufs=1)
                    zrp = wpool.tile([128, 512], F32, tag="zrp", bufs=1)
                    zrm = wpool.tile([128, 512], F32, tag="zrm", bufs=1)
                    nc.scalar.activation(feR[0:M], pfer[0:M], AF.Copy, scale=sc_ap)
                    nc.vector.scalar_tensor_tensor(
                        zrp[0:M], pfor[0:M], sc_ap, feR[0:M], op0=AT.mult, op1=AT.add)
                    nc.vector.scalar_tensor_tensor(
                        zrm[0:M], pfor[0:M], sc_ap, feR[0:M], op0=AT.mult, op1=AT.subtract)
                    pnei = ps.tile([128, 512], F32, tag="pa")
                    pnoi = ps.tile([128, 512], F32, tag="pb")
                    for k in range(4):
                        st, sp = (k == 0), (k == 3)
                        er = Ar[k][:, u0 : u0 + M]
                        ei = Ai[k][:, u0 : u0 + M]
                        orr = Ar[4 + k][:, u0 : u0 + M]
                        oi = Ai[4 + k][:, u0 : u0 + M]
                        if mu < 4:
                            nc.tensor.matmul(pnei[0:M], ei, CeT[k][:], start=st, stop=False)
                            nc.tensor.matmul(pnei[0:M], er, SeT[k][:], start=False, stop=sp,
                                             skip_group_check=True)
                            nc.tensor.matmul(pnoi[0:M], oi, CoT[k][:], start=st, stop=False)
                            nc.tensor.matmul(pnoi[0:M], orr, SoT[k][:], start=False, stop=sp,
                                             skip_group_check=True)
                        else:
                            nc.tensor.matmul(pnei[0:M], er, SeT[k][:], start=st, stop=sp)
                            nc.tensor.matmul(pnoi[0:M], orr, SoT[k][:], start=st, stop=sp)
                    feI = wpool.tile([128, 512], F32, tag="feI", bufs=1)
                    zip_ = wpool.tile([128, 512], F32, tag="zip", bufs=1)
                    zim = wpool.tile([128, 512], F32, tag="zim", bufs=1)
                    nc.scalar.activation(feI[0:M], pnei[0:M], AF.Copy, scale=sc_ap)
                    nc.vector.scalar_tensor_tensor(
                        zip_[0:M], pnoi[0:M], sc_ap, feI[0:M], op0=AT.mult, op1=AT.add)
                    nc.vector.scalar_tensor_tensor(
                        zim[0:M], pnoi[0:M], sc_ap, feI[0:M], op0=AT.mult, op1=AT.subtract)

                    # U+/- = |sc*Z|^2; squares on ACT, adds on DVE
                    up, um = zrp, zrm
                    nc.scalar.activation(up[0:M], zrp[0:M], AF.Square)
                    nc.scalar.activation(zip_[0:M], zip_[0:M], AF.Square)
                    nc.vector.tensor_tensor(out=up[0:M], in0=up[0:M], in1=zip_[0:M], op=AT.add)
                    nc.scalar.activation(zrm[0:M], zrm[0:M], AF.Square)
                    nc.scalar.activation(zim[0:M], zim[0:M], AF.Square)
                    nc.vector.tensor_tensor(out=um[0:M], in0=um[0:M], in1=zim[0:M], op=AT.add)
                    if mu == 0:
                        # zero DC: kills the catastrophic p2/q2 cancellation
                        nc.vector.tensor_copy(up[0:1, 0:1], zt[0:1, 0:1])
                    # fold to G strip: G[w']=U+(w')+U-(512-w')
                    nc.vector.tensor_tensor(
                        out=Gq[mu][0:M, 1:512], in0=up[0:M, 1:512],
                        in1=um[0:M, 511:0:-1], op=AT.add)
                    nc.vector.tensor_copy(Gq[mu][0:M, 0:1], up[0:M, 0:1])
                    nc.vector.tensor_copy(Gq[mu][0:M, 512:513], um[0:M, 0:1])
                    # p2 accumulation
                    rsp = wpool.tile([128, 1], F32, tag="rsp")
                    rsm = wpool.tile([128, 1], F32, tag="rsm")
                    nc.scalar.activation(up[0:M], up[0:M], AF.Square, accum_out=rsp[0:M])
                    nc.scalar.activation(um[0:M], um[0:M], AF.Square, accum_out=rsm[0:M])
                    nc.vector.scalar_tensor_tensor(
                        p2acc[0:M, 0:1], rsp[0:M], sw_t[0:M, 2 * mu + 1 : 2 * mu + 2],
                        p2acc[0:M, 0:1], op0=AT.mult, op1=AT.add)
                    nc.vector.scalar_tensor_tensor(
                        p2acc[0:M, 0:1], rsm[0:M], sw_t[0:M, 2 * mu + 1 : 2 * mu + 2],
                        p2acc[0:M, 0:1], op0=AT.mult, op1=AT.add)

            def dfrow(img, ci):
                """diagonal fold G + G^T (upper triangle) via PSUM accumulate,
                writing the bf16 barrel strips directly; one destination strip"""
                xb = Xb[img]
                if True:
                    for cj in range(ci, 4):
                        tp = ps.tile([128, 128], F32, tag="pd")
                        if ci == cj:
                            nc.tensor.matmul(tp[:], ident[:],
                                             Gq[ci][:, 128 * cj : 128 * cj + 128],
                                             start=True, stop=False)
                            nc.tensor.matmul(tp[:], Gq[cj][:, 128 * ci : 128 * ci + 128],
                                             ident[:], is_transpose=True,
                                             start=False, stop=True,
                                             skip_group_check=True)
                            nc.vector.tensor_tensor(
                                out=xb[ci][:, 128 * cj : 128 * cj + 128],
                                in0=tp[:], in1=mfd_t[:], op=AT.mult)
                        else:
                            nc.tensor.matmul(tp[:], Gq[cj][:, 128 * ci : 128 * ci + 128],
                                             ident[:], is_transpose=True,
                                             start=True, stop=True)
                            nc.vector.tensor_tensor(
                                out=xb[ci][:, 128 * cj : 128 * cj + 128],
                                in0=tp[:], in1=Gq[ci][:, 128 * cj : 128 * cj + 128],
                                op=AT.add)
                    # column 512 += transpose of strip-4 block
                    tp4 = ps.tile([128, 128], F32, tag="pd")
                    nc.tensor.matmul(tp4[:], Gq[4][:, 128 * ci : 128 * ci + 128],
                                     ident[:], is_transpose=True,
                                     start=True, stop=True)
                    nc.vector.tensor_tensor(
                        out=xb[ci][:, 512:513], in0=tp4[:, 0:1],
                        in1=Gq[ci][:, 512:513], op=AT.add)

            def brlstrip(img, ci):
                """barrel shear: align each row's columns to radial bins"""
                xb = Xb[img]
                if True:
                    off = 0
                    for t in range(NROUNDS):
                        lo, hi, m = table[ci][t]
                        wdt = hi - lo
                        if wdt <= 0:
                            continue
                        bit = 1 << t
                        gps = ci >= 3
                        eng = nc.gpsimd if gps else nc.vector
                        tmp = wpool.tile([128, 192 if gps else 512], BF16,
                                         tag=("btmpg" if gps else "btmp"), bufs=1)
                        eng.tensor_tensor(
                            out=tmp[:, 0:wdt], in0=xb[ci][:, lo:hi],
                            in1=bm_t[ci][:, off : off + wdt], op=AT.mult)
                        eng.tensor_tensor(
                            out=xb[ci][:, lo:hi], in0=xb[ci][:, lo:hi],
                            in1=tmp[:, 0:wdt], op=AT.subtract)
                        eng.tensor_tensor(
                            out=xb[ci][:, lo + bit : hi + bit],
                            in0=xb[ci][:, lo + bit : hi + bit],
                            in1=tmp[:, 0:wdt], op=AT.add)
                        off += wdt

            def red(img):
                """per-bin sums -> q2; loss = p2 - q2"""
                xb = Xb[img]
                ps_lo = ps.tile([1, 512], F32, tag="pa")
                ps_hi = ps.tile([1, 216], F32, tag="pb")
                for j, ci in enumerate((4, 3, 2, 1, 0)):
                    st, sp = (j == 0), (j == 4)
                    nc.tensor.matmul(ps_lo[:], onesb[:], xb[ci][:, 0:512], start=st, stop=sp)
                    nc.tensor.matmul(ps_hi[:], onesb[:], xb[ci][:, 512:WB], start=st, stop=sp)
                ssq = wpool.tile([1, WB], F32, tag="ssq", bufs=1)
                nc.scalar.activation(ssq[0:1, 0:512], ps_lo[:], AF.Square)
                nc.scalar.activation(ssq[0:1, 512:WB], ps_hi[:], AF.Square)
                nc.vector.tensor_tensor(out=ssq[:], in0=ssq[:], in1=ic_t[:], op=AT.mult)
                q2 = wpool.tile([1, 1], F32, tag="q2")
                nc.vector.tensor_reduce(q2[:], ssq[:], axis=mybir.AxisListType.X, op=AT.add)
                psp = ps.tile([1, 8], F32, tag="pd")
                nc.tensor.matmul(psp[:], ones[:], P2[img][:], start=True, stop=True)
                nc.vector.tensor_tensor(
                    out=lossv[0:1, img : img + 1], in0=psp[0:1, 0:1], in1=q2[:],
                    op=AT.subtract)

            def s2df(img):
                """s2 + fold + barrel, v-blocks descending so each strip's
                barrel overlaps the remaining blocks' matmuls"""
                s2pre(img)
                s2row(img, 4)
                # strip 4: the lone (512,512) diagonal cell keeps weight 1
                nc.scalar.activation(Xb[img][4][0:1, 512:513],
                                     Gq[4][0:1, 512:513], AF.Copy)
                brlstrip(img, 4)
                for mu in (3, 2, 1, 0):
                    s2row(img, mu)
                    dfrow(img, mu)
                    brlstrip(img, mu)

            # ---------------- program (software-pipelined over 2 images)
            # PE warmup: keep HAM un-throttled while the input/const DMAs
            # stream in (PE would otherwise idle ~20us and start cold)
            for i in range(96):
                pw = ps.tile([128, 128], F32, tag="pd")
                nc.tensor.matmul(pw[:], ident[:], ident[:], start=True, stop=True)

            for k in range(8):
                nc.sync.dma_start(Xt[k][:], x_p[0, 128 * k : 128 * k + 128, :])
            for k in range(8):
                nc.sync.dma_start(Cm_t[k][:], cm_p[128 * k : 128 * k + 128, :])
                nc.sync.dma_start(Sm_t[k][:], sm_p[128 * k : 128 * k + 128, :])
            nc.sync.dma_start(sw_t[:], sw_p[:])
            nc.sync.dma_start(ic_t[:], ic_p[:])
            for k in range(4):
                sl = slice(128 * k, 128 * k + 128)
                nc.sync.dma_start(CeT[k][:], ce_p[sl, :])
                nc.sync.dma_start(SeT[k][:], se_p[sl, :])
                nc.sync.dma_start(SenT[k][:], sen_p[sl, :])
                nc.sync.dma_start(CoT[k][:], co_p[sl, :])
                nc.sync.dma_start(SoT[k][:], so_p[sl, :])
                nc.sync.dma_start(SonT[k][:], son_p[sl, :])

            sc = nc.named_scope("s1_0"); sc.__enter__()
            s1(0)
            sc.__exit__(None, None, None)
            # prefetch image 1 and the barrel masks behind the s1 constants
            for k in range(8):
                nc.sync.dma_start(Xt[k][:], x_p[1, 128 * k : 128 * k + 128, :])
            for ci in range(5):
                wci = sum(hi - lo for (lo, hi, m) in table[ci])
                if wci > 0:
                    nc.sync.dma_start(bm_t[ci][:, 0:wci],
                                      bm_p[128 * ci : 128 * ci + 128, 0:wci])
            nc.sync.dma_start(mfd_t[:], md_p[:])

            sc = nc.named_scope("s2_0"); sc.__enter__()
            s2df(0)
            sc.__exit__(None, None, None)
            sc = nc.named_scope("s1_1"); sc.__enter__()
            s1(1)
            sc.__exit__(None, None, None)
            sc = nc.named_scope("red_0"); sc.__enter__()
            red(0)
            sc.__exit__(None, None, None)
            sc = nc.named_scope("s2_1"); sc.__enter__()
            s2df(1)
            sc.__exit__(None, None, None)
            sc = nc.named_scope("red_1"); sc.__enter__()
            red(1)
            sc.__exit__(None, None, None)

            nc.sync.dma_start(out_p[:], lossv[:])

    nc.compile()
    return nc


def _get_nc():
    if "nc" not in _CACHE:
        _CACHE["nc"] = _build_nc()
    return _CACHE["nc"]


# ---------------------------------------------------------------- entry point
def kernel(prob_cg: np.ndarray) -> np.ndarray:
    hc = _host_constants()
    nc = _get_nc()
    x = prob_cg[:, 0, :, :].astype(np.float32)
    # pre-permute columns to [even m | odd m] so s1 writes even/odd A blocks
    xp = np.ascontiguousarray(
        np.concatenate([x[:, :, 0::2], x[:, :, 1::2]], axis=2))
    in_maps = []
    for i in range(N_CORES):
        in_maps.append(
            dict(
                x=xp[2 * i : 2 * i + 2],
                cm=hc["Cm"], sm=hc["Sm"],
                ce=hc["Ce"], se=hc["Se"], sen=hc["Sen"],
                co=hc["Co"], so=hc["So"], son=hc["Son"],
                bm=hc["bmask_bf16"], md=hc["mfd_bf16"],
                sw=hc["swc"], ic=hc["invc"],
            )
        )
    trace = os.environ.get("AT_TRACE", "0") == "1"
    kw = {}
    if trace and os.environ.get("AT_TMPDIR"):
        kw["tmpdir"] = os.environ["AT_TMPDIR"]
    res = run_bass_kernel_spmd(nc, in_maps, core_ids=list(range(N_CORES)), trace=trace, **kw)
    if trace and res.exec_time_ns is not None:
        print(f"HW exec time: {res.exec_time_ns} ns")
        if res.per_core_scope_times:
            for kname, v in sorted(res.per_core_scope_times.items()):
                print(f"  scope {kname}: {v}")
    losses = np.concatenate([r["out"].reshape(-1) for r in res.results])
    loss = losses.mean() + (H * H) * (EPS * EPS)
    return np.float32(WA * loss)


# revision 40
# speedup vs baseline: 1.0403x; 1.0007x over previous
"""AnisotropySuppressionLoss on 8 TRN2 NeuronCores (Bass/Tile), v2.

Per image (1024x1024 fp32):
  s1: A[m, v] = DFT_u x[u, m], v = 0..512 (real-input half spectrum),
      via f32r matmuls. Host pre-permutes image columns to [even m | odd m]
      so A lands in even/odd m' blocks.
  s2: radix-2 over m with the twiddle absorbed into the odd-m DFT matrix:
      Fe = DFT_{even m}(A), Fo' = sum_{odd m} A e^{-2pi i m w/1024};
      Z(w) = Fe + Fo', Z(w+512) = Fe - Fo' (w = 0..511). Halves matmul work
      vs a direct 1024-wide second DFT; butterflies are 4 DVE adds/block.
  power: U+/- = w_v/H^2 * |Z|^2 via ACT squares (scale) + DVE stt squares.
  fold:  G[v,w'] = U+(w') + U-(512-w') into bf16 strips; diagonal fold
      G + G^T via PE transpose accumulated in PSUM (ACT copy back).
  radial: barrel-shift shear (exact), bf16; strips 0-2 on DVE,
      3-4 on GpSimd; PE warmup stream covers the initial DMA window.
  loss_img = sum w*P^2/w - sum_b S_b^2/c_b  (+ H*W*eps^2 on host).
Data-parallel: batch 16 -> 2 images/core on 8 cores; host averages.
"""

import os
import sys

sys.path.insert(0, "/opt/trn_rl_repo")

import numpy as np

import concourse.bass as bass
import concourse.tile as tile
from concourse import bacc, mybir
from concourse.bass_utils import run_bass_kernel_spmd
from concourse.masks import make_identity

F32 = mybir.dt.float32
F32R = mybir.dt.float32r
BF16 = mybir.dt.bfloat16

H = 1024
NQ = 513          # quadrant size (|du|, |dv| in 0..512)
NB = 725          # radial bins 0..724
WB = 728          # barrel buffer width
NROUNDS = 8
N_CORES = 8
IMGS_PER_CORE = 2
WA = 0.002
EPS = 1e-12
CHUNKS = [(0, 128), (128, 256), (256, 384), (384, 512), (512, 513)]
AF = mybir.ActivationFunctionType

_CACHE = {}


# ---------------------------------------------------------------- host consts
def _gen_barrel_masks():
    """Per (chunk, round): (lo, hi, move_mask[128, hi-lo]) in quadrant coords.
    Cells (a, b>=a) carry delta = bin - b; each round moves cells with bit t
    set right by 2^t. Merges are exact (same remaining delta)."""
    rem = -np.ones((NQ, WB), dtype=np.int64)
    for a in range(NQ):
        cols = np.arange(a, NQ)
        bins = np.floor(np.sqrt(a * a + cols.astype(np.float64) ** 2)).astype(np.int64)
        rem[a, cols] = bins - cols
    table = [[] for _ in CHUNKS]
    for t in range(NROUNDS):
        bit = 1 << t
        move = (rem >= 0) & ((rem & bit) != 0)
        for ci, (c0, c1) in enumerate(CHUNKS):
            mv = move[c0:c1]
            cols_any = np.nonzero(mv.any(axis=0))[0]
            if len(cols_any) == 0:
                table[ci].append((0, 0, None))
            else:
                # widen span to even bounds: 4B-aligned bf16 slices let the
                # DVE pick its 2x packed mode
                lo, hi = int(cols_any[0]) & ~1, int(cols_any[-1]) + 1
                hi += hi & 1
                m = np.zeros((128, hi - lo), dtype=np.float32)
                m[: c1 - c0] = mv[:, lo:hi]
                table[ci].append((lo, hi, m))
        new_rem = -np.ones_like(rem)
        stay = (rem >= 0) & ~move
        new_rem[stay] = rem[stay]
        sr, sc = np.nonzero(move)
        dc = sc + bit
        landing = rem[sr, sc] - bit
        cur = new_rem[sr, dc]
        assert ((cur == -1) | (cur == landing)).all()
        new_rem[sr, dc] = landing
        rem = new_rem
    assert (rem[rem >= 0] == 0).all()
    return table


def _host_constants():
    if "consts" in _CACHE:
        return _CACHE["consts"]
    import ml_dtypes

    u = np.arange(H, dtype=np.float64)
    v = np.arange(520, dtype=np.float64)
    ang1 = 2.0 * np.pi * np.outer(u, v) / H
    Cm = np.cos(ang1).astype(np.float32)        # [1024, 520]
    Sm = np.sin(ang1).astype(np.float32)

    mp = np.arange(512, dtype=np.float64)
    w = np.arange(512, dtype=np.float64)
    ae = 2.0 * np.pi * np.outer(mp, w) / 512.0
    ao = 2.0 * np.pi * np.outer(2 * mp + 1, w) / 1024.0
    Ce = np.cos(ae).astype(np.float32)
    Se = np.sin(ae).astype(np.float32)
    Co = np.cos(ao).astype(np.float32)
    So = np.sin(ao).astype(np.float32)

    # radial bin counts exactly as reference._radial_bins (unshifted coords)
    y = np.minimum(np.arange(H), H - np.arange(H))
    yy, xx = np.meshgrid(y, y, indexing="ij")
    dist = np.sqrt((xx.astype(np.float64)) ** 2 + yy.astype(np.float64) ** 2)
    bins_full = np.clip(dist.astype(np.int32), 0, NB - 1)
    counts = np.bincount(bins_full.reshape(-1), minlength=NB).astype(np.float64)
    invc = np.zeros((1, WB), dtype=np.float32)
    invc[0, :NB] = (1.0 / counts).astype(np.float32)

    # row weights w_v for v = 0..512; sw cols: 2*mu = sqrt(w)/H (ACT square
    # scale), 2*mu+1 = 1/w (p2 accumulation), 10+mu = w/H^2 (DVE stt square)
    wv = np.full(NQ, 2.0)
    wv[0] = 1.0
    wv[512] = 1.0
    swc = np.zeros((128, 16), dtype=np.float32)
    for mu in range(5):
        c0, c1 = CHUNKS[mu]
        n = c1 - c0
        swc[:n, 2 * mu] = (np.sqrt(wv[c0:c1]) / H).astype(np.float32)
        swc[:n, 2 * mu + 1] = (1.0 / wv[c0:c1]).astype(np.float32)
        swc[:n, 10 + mu] = (-np.sqrt(wv[c0:c1]) / H).astype(np.float32)

    table = _gen_barrel_masks()
    chunk_w = [max(1, sum(hi - lo for (lo, hi, m) in table[ci])) for ci in range(5)]
    maxw = max(chunk_w)
    bmask = np.zeros((640, maxw), dtype=np.float32)
    for ci in range(5):
        off = 0
        for (lo, hi, m) in table[ci]:
            if m is None:
                continue
            bmask[128 * ci : 128 * ci + 128, off : off + hi - lo] = m
            off += hi - lo

    # diagonal-block mask: 0 below diag, 0.5 on diag, 1 above (block-local)
    a = np.arange(128)
    mfd = (a[None, :] > a[:, None]).astype(np.float32)
    mfd[a, a] = 0.5

    _CACHE["consts"] = dict(
        Cm=Cm, Sm=Sm, Ce=Ce, Se=Se, Sen=(-Se), Co=Co, So=So, Son=(-So),
        invc=invc, swc=swc,
        bmask_bf16=bmask.astype(ml_dtypes.bfloat16),
        mfd_bf16=mfd.astype(ml_dtypes.bfloat16),
        table=table, maxw=maxw, counts=counts,
    )
    return _CACHE["consts"]


# ---------------------------------------------------------------- device build
def _build_nc():
    hc = _host_constants()
    table, maxw = hc["table"], hc["maxw"]

    nc = bacc.Bacc("TRN2", target_bir_lowering=False, debug=False)
    x_p = nc.declare_dram_parameter("x", [IMGS_PER_CORE, H, H], F32R, isOutput=False)
    cm_p = nc.declare_dram_parameter("cm", [H, 520], F32R, isOutput=False)
    sm_p = nc.declare_dram_parameter("sm", [H, 520], F32R, isOutput=False)
    ce_p = nc.declare_dram_parameter("ce", [512, 512], F32R, isOutput=False)
    se_p = nc.declare_dram_parameter("se", [512, 512], F32R, isOutput=False)
    sen_p = nc.declare_dram_parameter("sen", [512, 512], F32R, isOutput=False)
    co_p = nc.declare_dram_parameter("co", [512, 512], F32R, isOutput=False)
    so_p = nc.declare_dram_parameter("so", [512, 512], F32R, isOutput=False)
    son_p = nc.declare_dram_parameter("son", [512, 512], F32R, isOutput=False)
    bm_p = nc.declare_dram_parameter("bm", [640, maxw], BF16, isOutput=False)
    md_p = nc.declare_dram_parameter("md", [128, 128], BF16, isOutput=False)
    sw_p = nc.declare_dram_parameter("sw", [128, 16], F32, isOutput=False)
    ic_p = nc.declare_dram_parameter("ic", [1, WB], F32, isOutput=False)
    out_p = nc.declare_dram_parameter("out", [1, IMGS_PER_CORE], F32, isOutput=True)

    AT = mybir.AluOpType

    with tile.TileContext(nc) as tc:
        with (
            tc.tile_pool(name="const", bufs=1) as cpool,
            tc.tile_pool(name="xin", bufs=1) as xpool,
            tc.tile_pool(name="arr", bufs=1) as apool,
            tc.tile_pool(name="quad", bufs=1) as qpool,
            tc.tile_pool(name="work", bufs=2) as wpool,
            tc.tile_pool(name="ps", bufs=2, space="PSUM") as ps,
        ):
            # ---------------- constants
            Cm_t = [cpool.tile([128, 520], F32R, tag=f"cm{k}", name=f"cm{k}") for k in range(8)]
            Sm_t = [cpool.tile([128, 520], F32R, tag=f"sm{k}", name=f"sm{k}") for k in range(8)]
            CeT = [cpool.tile([128, 512], F32R, tag=f"ce{k}", name=f"ce{k}") for k in range(4)]
            SeT = [cpool.tile([128, 512], F32R, tag=f"sE{k}", name=f"sE{k}") for k in range(4)]
            SenT = [cpool.tile([128, 512], F32R, tag=f"sn{k}", name=f"sn{k}") for k in range(4)]
            CoT = [cpool.tile([128, 512], F32R, tag=f"co{k}", name=f"co{k}") for k in range(4)]
            SoT = [cpool.tile([128, 512], F32R, tag=f"sO{k}", name=f"sO{k}") for k in range(4)]
            SonT = [cpool.tile([128, 512], F32R, tag=f"sm{k}b", name=f"sm{k}b") for k in range(4)]
            bm_t = [
                cpool.tile([128, max(1, sum(hi - lo for (lo, hi, m) in table[ci]))],
                           BF16, tag=f"bm{ci}", name=f"bm{ci}")
                for ci in range(5)
            ]
            mfd_t = cpool.tile([128, 128], BF16, tag="mfd")
            sw_t = cpool.tile([128, 16], F32, tag="sw")
            ic_t = cpool.tile([1, WB], F32, tag="ic")
            ident = cpool.tile([128, 128], F32, tag="ident")
            make_identity(nc, ident[:])
            identr = cpool.tile([128, 128], F32R, tag="identr")
            nc.vector.tensor_copy(ident[:], ident[:])
            ones32 = cpool.tile([128, 1], F32, tag="ones32")
            nc.gpsimd.memset(ones32[:], 1.0)
            ones = cpool.tile([128, 1], F32R, tag="ones")
            nc.vector.tensor_copy(ones[:], ones32[:])
            onesb = cpool.tile([128, 1], BF16, tag="onesb")
            nc.vector.tensor_copy(onesb[:], ones32[:])
            zt = cpool.tile([128, 8], F32, tag="zt")
            nc.gpsimd.memset(zt[:], 0.0)
            lossv = cpool.tile([1, IMGS_PER_CORE], F32, tag="lossv")

            # ---------------- per-image persistent arrays
            Xt = [xpool.tile([128, H], F32R, tag=f"x{k}", name=f"x{k}") for k in range(8)]
            # A blocks: j=0..3 even m' chunks, j=4..7 odd m' chunks
            Ar = [apool.tile([128, NQ], F32R, tag=f"ar{j}", name=f"ar{j}") for j in range(8)]
            Ai = [apool.tile([128, NQ], F32R, tag=f"ai{j}", name=f"ai{j}") for j in range(8)]
            # f32r folded-G strips (shared across images) + bf16 barrel strips
            # double-buffered across images (for the pipelined red)
            Gq = [qpool.tile([128, NQ], F32, tag=f"gq{ci}", name=f"gq{ci}")
                  for ci in range(5)]
            Xb = [[qpool.tile([128, WB], BF16, tag=f"xb{p}_{ci}", name=f"xb{p}_{ci}")
                   for ci in range(5)] for p in range(IMGS_PER_CORE)]
            P2 = [qpool.tile([128, 8], F32R, tag=f"p2acc{p}", name=f"p2acc{p}")
                  for p in range(IMGS_PER_CORE)]

            def s1(img):
                """first DFT: fills Ar/Ai blocks; Ai = -Im(A)"""
                for m in range(8):
                    pr_lo = ps.tile([128, 512], F32, tag="pa", bufs=3)
                    pr_hi = ps.tile([128, 8], F32, tag="pd")
                    pt_lo = ps.tile([128, 512], F32, tag="pb", bufs=3)
                    for k in range(8):
                        lhs = Xt[k][:, 128 * m : 128 * m + 128]
                        st, sp = (k == 0), (k == 7)
                        nc.tensor.matmul(pr_lo[:], lhs, Cm_t[k][:, 0:512], start=st, stop=sp)
                        nc.tensor.matmul(pr_hi[:], lhs, Cm_t[k][:, 512:520], start=st, stop=sp)
                        nc.tensor.matmul(pt_lo[:], lhs, Sm_t[k][:, 0:512], start=st, stop=sp)
                    nc.scalar.activation(Ar[m][:, 0:512], pr_lo[:], AF.Copy)
                    nc.scalar.activation(Ar[m][:, 512:513], pr_hi[:, 0:1], AF.Copy)
                    nc.scalar.activation(Ai[m][:, 0:512], pt_lo[:], AF.Copy)
                    nc.vector.tensor_copy(Ai[m][:, 512:513], zt[:, 0:1])

            def s2pre(img):
                """zero barrel strips, the strip-4 G row, and p2acc"""
                xb = Xb[img]
                for ci in range(5):
                    nc.gpsimd.memset(xb[ci][:], 0.0)
                nc.gpsimd.memset(Gq[4][:], 0.0)
                nc.vector.tensor_copy(P2[img][:], zt[:])

            def s2row(img, mu):
                """second DFT (even/odd split) + power + fold, one v-block"""
                p2acc = P2[img]
                if True:
                    M = 128 if mu < 4 else 1
                    u0 = 128 * mu
                    pfer = ps.tile([128, 512], F32, tag="pa", bufs=3)
                    pfor = ps.tile([128, 512], F32, tag="pb", bufs=3)
                    for k in range(4):
                        st, sp = (k == 0), (k == 3)
                        er = Ar[k][:, u0 : u0 + M]
                        ei = Ai[k][:, u0 : u0 + M]
                        orr = Ar[4 + k][:, u0 : u0 + M]
                        oi = Ai[4 + k][:, u0 : u0 + M]
                        if mu < 4:
                            nc.tensor.matmul(pfer[0:M], er, CeT[k][:], start=st, stop=False)
                            nc.tensor.matmul(pfer[0:M], ei, SenT[k][:], start=False, stop=sp,
                                             skip_group_check=True)
                            nc.tensor.matmul(pfor[0:M], orr, CoT[k][:], start=st, stop=False)
                            nc.tensor.matmul(pfor[0:M], oi, SonT[k][:], start=False, stop=sp,
                                             skip_group_check=True)
                        else:
                            nc.tensor.matmul(pfer[0:M], er, CeT[k][:], start=st, stop=sp)
                            nc.tensor.matmul(pfor[0:M], orr, CoT[k][:], start=st, stop=sp)
                    sc_ap = sw_t[0:M, 2 * mu : 2 * mu + 1]
                    feR = wpool.tile([128, 512], F32, tag="feR", bufs=1)
                    zrp = wpool.tile([128, 512], F32, tag="zrp", bufs=1)
                    zrm = wpool.tile([128, 512], F32, tag="zrm", bufs=1)
                    nc.scalar.activation(feR[0:M], pfer[0:M], AF.Copy, scale=sc_ap)
                    nc.vector.scalar_tensor_tensor(
                        zrp[0:M], pfor[0:M], sc_ap, feR[0:M], op0=AT.mult, op1=AT.add)
                    nc.vector.scalar_tensor_tensor(
                        zrm[0:M], pfor[0:M], sc_ap, feR[0:M], op0=AT.mult, op1=AT.subtract)
                    pnei = ps.tile([128, 512], F32, tag="pa", bufs=3)
                    pnoi = ps.tile([128, 512], F32, tag="pb", bufs=3)
                    for k in range(4):
                        st, sp = (k == 0), (k == 3)
                        er = Ar[k][:, u0 : u0 + M]
                        ei = Ai[k][:, u0 : u0 + M]
                        orr = Ar[4 + k][:, u0 : u0 + M]
                        oi = Ai[4 + k][:, u0 : u0 + M]
                        if mu < 4:
                            nc.tensor.matmul(pnei[0:M], ei, CeT[k][:], start=st, stop=False)
                            nc.tensor.matmul(pnei[0:M], er, SeT[k][:], start=False, stop=sp,
                                             skip_group_check=True)
                            nc.tensor.matmul(pnoi[0:M], oi, CoT[k][:], start=st, stop=False)
                            nc.tensor.matmul(pnoi[0:M], orr, SoT[k][:], start=False, stop=sp,
                                             skip_group_check=True)
                        else:
                            nc.tensor.matmul(pnei[0:M], er, SeT[k][:], start=st, stop=sp)
                            nc.tensor.matmul(pnoi[0:M], orr, SoT[k][:], start=st, stop=sp)
                    feI = wpool.tile([128, 512], F32, tag="feI", bufs=1)
                    zip_ = wpool.tile([128, 512], F32, tag="zip", bufs=1)
                    zim = wpool.tile([128, 512], F32, tag="zim", bufs=1)
                    nc.scalar.activation(feI[0:M], pnei[0:M], AF.Copy, scale=sc_ap)
                    nc.vector.scalar_tensor_tensor(
                        zip_[0:M], pnoi[0:M], sc_ap, feI[0:M], op0=AT.mult, op1=AT.add)
                    nc.vector.scalar_tensor_tensor(
                        zim[0:M], pnoi[0:M], sc_ap, feI[0:M], op0=AT.mult, op1=AT.subtract)

                    # U+/- = |sc*Z|^2; squares on ACT, adds on DVE
                    up, um = zrp, zrm
                    nc.scalar.activation(up[0:M], zrp[0:M], AF.Square)
                    nc.scalar.activation(zip_[0:M], zip_[0:M], AF.Square)
                    nc.vector.tensor_tensor(out=up[0:M], in0=up[0:M], in1=zip_[0:M], op=AT.add)
                    nc.scalar.activation(zrm[0:M], zrm[0:M], AF.Square)
                    nc.scalar.activation(zim[0:M], zim[0:M], AF.Square)
                    nc.vector.tensor_tensor(out=um[0:M], in0=um[0:M], in1=zim[0:M], op=AT.add)
                    if mu == 0:
                        # zero DC: kills the catastrophic p2/q2 cancellation
                        nc.vector.tensor_copy(up[0:1, 0:1], zt[0:1, 0:1])
                    # fold to G strip: G[w']=U+(w')+U-(512-w')
                    nc.vector.tensor_tensor(
                        out=Gq[mu][0:M, 1:512], in0=up[0:M, 1:512],
                        in1=um[0:M, 511:0:-1], op=AT.add)
                    nc.vector.tensor_copy(Gq[mu][0:M, 0:1], up[0:M, 0:1])
                    nc.vector.tensor_copy(Gq[mu][0:M, 512:513], um[0:M, 0:1])
                    # p2 accumulation
                    rsp = wpool.tile([128, 1], F32, tag="rsp")
                    rsm = wpool.tile([128, 1], F32, tag="rsm")
                    nc.scalar.activation(up[0:M], up[0:M], AF.Square, accum_out=rsp[0:M])
                    nc.scalar.activation(um[0:M], um[0:M], AF.Square, accum_out=rsm[0:M])
                    nc.vector.scalar_tensor_tensor(
                        p2acc[0:M, 0:1], rsp[0:M], sw_t[0:M, 2 * mu + 1 : 2 * mu + 2],
                        p2acc[0:M, 0:1], op0=AT.mult, op1=AT.add)
                    nc.vector.scalar_tensor_tensor(
                        p2acc[0:M, 0:1], rsm[0:M], sw_t[0:M, 2 * mu + 1 : 2 * mu + 2],
                        p2acc[0:M, 0:1], op0=AT.mult, op1=AT.add)

            def dfrow(img, ci):
                """diagonal fold G + G^T (upper triangle) via PSUM accumulate,
                writing the bf16 barrel strips directly; one destination strip"""
                xb = Xb[img]
                if True:
                    for cj in range(ci, 4):
                        tp = ps.tile([128, 128], F32, tag="pd")
                        if ci == cj:
                            nc.tensor.matmul(tp[:], ident[:],
                                             Gq[ci][:, 128 * cj : 128 * cj + 128],
                                             start=True, stop=False)
                            nc.tensor.matmul(tp[:], Gq[cj][:, 128 * ci : 128 * ci + 128],
                                             ident[:], is_transpose=True,
                                             start=False, stop=True,
                                             skip_group_check=True)
                            nc.vector.tensor_tensor(
                                out=xb[ci][:, 128 * cj : 128 * cj + 128],
                                in0=tp[:], in1=mfd_t[:], op=AT.mult)
                        else:
                            nc.tensor.matmul(tp[:], Gq[cj][:, 128 * ci : 128 * ci + 128],
                                             ident[:], is_transpose=True,
                                             start=True, stop=True)
                            nc.vector.tensor_tensor(
                                out=xb[ci][:, 128 * cj : 128 * cj + 128],
                                in0=tp[:], in1=Gq[ci][:, 128 * cj : 128 * cj + 128],
                                op=AT.add)
                    # column 512 += transpose of strip-4 block
                    tp4 = ps.tile([128, 128], F32, tag="pd")
                    nc.tensor.matmul(tp4[:], Gq[4][:, 128 * ci : 128 * ci + 128],
                                     ident[:], is_transpose=True,
                                     start=True, stop=True)
                    nc.vector.tensor_tensor(
                        out=xb[ci][:, 512:513], in0=tp4[:, 0:1],
                        in1=Gq[ci][:, 512:513], op=AT.add)

            def brlstrip(img, ci):
                """barrel shear: align each row's columns to radial bins"""
                xb = Xb[img]
                if True:
                    off = 0
                    for t in range(NROUNDS):
                        lo, hi, m = table[ci][t]
                        wdt = hi - lo
                        if wdt <= 0:
                            continue
                        bit = 1 << t
                        gps = ci >= 3
                        eng = nc.gpsimd if gps else nc.vector
                        tmp = wpool.tile([128, 192 if gps else 512], BF16,
                                         tag=("btmpg" if gps else "btmp"), bufs=1)
                        eng.tensor_tensor(
                            out=tmp[:, 0:wdt], in0=xb[ci][:, lo:hi],
                            in1=bm_t[ci][:, off : off + wdt], op=AT.mult)
                        eng.tensor_tensor(
                            out=xb[ci][:, lo:hi], in0=xb[ci][:, lo:hi],
                            in1=tmp[:, 0:wdt], op=AT.subtract)
                        eng.tensor_tensor(
                            out=xb[ci][:, lo + bit : hi + bit],
                            in0=xb[ci][:, lo + bit : hi + bit],
                            in1=tmp[:, 0:wdt], op=AT.add)
                        off += wdt

            def red(img):
                """per-bin sums -> q2; loss = p2 - q2"""
                xb = Xb[img]
                ps_lo = ps.tile([1, 512], F32, tag="pa", bufs=3)
                ps_hi = ps.tile([1, 216], F32, tag="pb", bufs=3)
                for j, ci in enumerate((4, 3, 2, 1, 0)):
                    st, sp = (j == 0), (j == 4)
                    nc.tensor.matmul(ps_lo[:], onesb[:], xb[ci][:, 0:512], start=st, stop=sp)
                    nc.tensor.matmul(ps_hi[:], onesb[:], xb[ci][:, 512:WB], start=st, stop=sp)
                ssq = wpool.tile([1, WB], F32, tag="ssq", bufs=1)
                nc.scalar.activation(ssq[0:1, 0:512], ps_lo[:], AF.Square)
                nc.scalar.activation(ssq[0:1, 512:WB], ps_hi[:], AF.Square)
                nc.vector.tensor_tensor(out=ssq[:], in0=ssq[:], in1=ic_t[:], op=AT.mult)
                q2 = wpool.tile([1, 1], F32, tag="q2")
                nc.vector.tensor_reduce(q2[:], ssq[:], axis=mybir.AxisListType.X, op=AT.add)
                psp = ps.tile([1, 8], F32, tag="pd")
                nc.tensor.matmul(psp[:], ones[:], P2[img][:], start=True, stop=True)
                nc.vector.tensor_tensor(
                    out=lossv[0:1, img : img + 1], in0=psp[0:1, 0:1], in1=q2[:],
                    op=AT.subtract)

            def s2df(img):
                """s2 + fold + barrel, v-blocks descending so each strip's
                barrel overlaps the remaining blocks' matmuls"""
                s2pre(img)
                s2row(img, 4)
                # strip 4: the lone (512,512) diagonal cell keeps weight 1
                nc.scalar.activation(Xb[img][4][0:1, 512:513],
                                     Gq[4][0:1, 512:513], AF.Copy)
                brlstrip(img, 4)
                for mu in (3, 2, 1, 0):
                    s2row(img, mu)
                    dfrow(img, mu)
                    brlstrip(img, mu)

            # ---------------- program (software-pipelined over 2 images)
            # PE warmup: keep HAM un-throttled while the input/const DMAs
            # stream in (PE would otherwise idle ~20us and start cold)
            for i in range(96):
                pw = ps.tile([128, 128], F32, tag="pd")
                nc.tensor.matmul(pw[:], ident[:], ident[:], start=True, stop=True)

            for k in range(8):
                nc.sync.dma_start(Xt[k][:], x_p[0, 128 * k : 128 * k + 128, :])
            for k in range(8):
                nc.sync.dma_start(Cm_t[k][:], cm_p[128 * k : 128 * k + 128, :])
                nc.sync.dma_start(Sm_t[k][:], sm_p[128 * k : 128 * k + 128, :])
            nc.sync.dma_start(sw_t[:], sw_p[:])
            nc.sync.dma_start(ic_t[:], ic_p[:])
            for k in range(4):
                sl = slice(128 * k, 128 * k + 128)
                nc.sync.dma_start(CeT[k][:], ce_p[sl, :])
                nc.sync.dma_start(SeT[k][:], se_p[sl, :])
                nc.sync.dma_start(SenT[k][:], sen_p[sl, :])
                nc.sync.dma_start(CoT[k][:], co_p[sl, :])
                nc.sync.dma_start(SoT[k][:], so_p[sl, :])
                nc.sync.dma_start(SonT[k][:], son_p[sl, :])

            sc = nc.named_scope("s1_0"); sc.__enter__()
            s1(0)
            sc.__exit__(None, None, None)
            # prefetch image 1 and the barrel masks behind the s1 constants
            for k in range(8):
                nc.sync.dma_start(Xt[k][:], x_p[1, 128 * k : 128 * k + 128, :])
            for ci in range(5):
                wci = sum(hi - lo for (lo, hi, m) in table[ci])
                if wci > 0:
                    nc.sync.dma_start(bm_t[ci][:, 0:wci],
                                      bm_p[128 * ci : 128 * ci + 128, 0:wci])
            nc.sync.dma_start(mfd_t[:], md_p[:])

            sc = nc.named_scope("s2_0"); sc.__enter__()
            s2df(0)
            sc.__exit__(None, None, None)
            sc = nc.named_scope("s1_1"); sc.__enter__()
            s1(1)
            sc.__exit__(None, None, None)
            sc = nc.named_scope("red_0"); sc.__enter__()
            red(0)
            sc.__exit__(None, None, None)
            sc = nc.named_scope("s2_1"); sc.__enter__()
            s2df(1)
            sc.__exit__(None, None, None)
            sc = nc.named_scope("red_1"); sc.__enter__()
            red(1)
            sc.__exit__(None, None, None)

            nc.sync.dma_start(out_p[:], lossv[:])

    nc.compile()
    return nc


def _get_nc():
    if "nc" not in _CACHE:
        _CACHE["nc"] = _build_nc()
    return _CACHE["nc"]


# ---------------------------------------------------------------- entry point
def kernel(prob_cg: np.ndarray) -> np.ndarray:
    hc = _host_constants()
    nc = _get_nc()
    x = prob_cg[:, 0, :, :].astype(np.float32)
    # pre-permute columns to [even m | odd m] so s1 writes even/odd A blocks
    xp = np.ascontiguousarray(
        np.concatenate([x[:, :, 0::2], x[:, :, 1::2]], axis=2))
    in_maps = []
    for i in range(N_CORES):
        in_maps.append(
            dict(
                x=xp[2 * i : 2 * i + 2],
                cm=hc["Cm"], sm=hc["Sm"],
                ce=hc["Ce"], se=hc["Se"], sen=hc["Sen"],
                co=hc["Co"], so=hc["So"], son=hc["Son"],
                bm=hc["bmask_bf16"], md=hc["mfd_bf16"],
                sw=hc["swc"], ic=hc["invc"],
            )
        )
    trace = os.environ.get("AT_TRACE", "0") == "1"
    kw = {}
    if trace and os.environ.get("AT_TMPDIR"):
        kw["tmpdir"] = os.environ["AT_TMPDIR"]
    res = run_bass_kernel_spmd(nc, in_maps, core_ids=list(range(N_CORES)), trace=trace, **kw)
    if trace and res.exec_time_ns is not None:
        print(f"HW exec time: {res.exec_time_ns} ns")
        if res.per_core_scope_times:
            for kname, v in sorted(res.per_core_scope_times.items()):
                print(f"  scope {kname}: {v}")
    losses = np.concatenate([r["out"].reshape(-1) for r in res.results])
    loss = losses.mean() + (H * H) * (EPS * EPS)
    return np.float32(WA * loss)
